# revision 1
# baseline (speedup 1.0000x reference)
"""Block-sparse causal self-attention on 8 TRN2 NeuronCores (SPMD Bass/Tile kernel).

Sharding: core c -> (batch b = c//2, head-group g = c%2 of 6 heads).
Each core computes qkv projection (its 6 heads), masked attention, and a
partial output projection (its 384 rows of W_proj).  Host sums the two
partials per batch and concatenates batches.

Token reorder (host-side permutation, inverted on output):
  [U_0 .. U_7 | A]  with U_i = [tactile_i (16), image_i (196)], A = 9 actions.
This makes the block-sparse mask nearly block-lower-triangular with
frame-aligned boundaries, so most 128-wide key tiles are either fully
visible or fully masked; the few partial tiles get an elementwise
multiply restricted to the bounding box of their masked region.

Attention is computed in transposed layout S^T[k, q] so that softmax
normalization comes from a ones-column appended to V (rowsum lands in the
PV matmul output) and no on-chip transposes are needed anywhere.
All matmuls run as float32r (TF32-like, 1 cycle/row for N>=256).
"""

import os
import sys
from contextlib import ExitStack

import numpy as np

for _p in ("/opt/trn_rl_repo", "/root/.axon_site/_ro/trn_rl_repo"):
    if os.path.isdir(_p) and _p not in sys.path:
        sys.path.insert(0, _p)

import concourse.bass as bass
import concourse.tile as tile
from concourse import mybir
from concourse.bass_utils import run_bass_kernel_spmd

F32 = mybir.dt.float32
F32R = mybir.dt.float32r
AF = mybir.ActivationFunctionType

L, PP, PT = 8, 196, 16
T, C, NH, B, HD = 1705, 768, 12, 4, 64
NCORES = 8
NHG = NH // 2          # heads per core = 6
NPACK = NHG // 2       # head pairs per core = 3
KC = C // 128          # 6 contraction tiles over C
KT = 128               # key tile size
NKT = (T + KT - 1) // KT   # 14
TP = 1706              # T padded to even (fp32r needs even free sizes)
KWP = 42               # padded tail k-tile width (fp32r: stationary M even)
# frame-aligned query chunks in permuted order [U_0..U_7 | A]
QCH = [(0, 424), (424, 848), (848, 1272), (1272, T)]
QCHC = [(0, 424), (424, 848), (848, 1272), (1272, TP)]  # compute chunks (even n)
HD1 = HD + 2           # V width: 64 V cols | ones col | zero pad col


def _perm():
    idx = []
    for i in range(L):
        idx += list(range(9 + PT * i, 9 + PT * (i + 1)))
        idx += list(range(9 + L * PT + PP * i, 9 + L * PT + PP * (i + 1)))
    idx += list(range(0, 9))
    return np.asarray(idx, dtype=np.int64)


PERM = _perm()


def _analyze(mask_perm):
    """Compile-time plan from the (permuted) boolean mask.

    Returns (plan, mpack):
      plan: per query-chunk, list of (kt, kw, bbox-or-None, moff) where bbox =
            (r0, r1, c0, c1) of the masked (zero) region inside the S^T tile
            [kw keys x chunk queries]; moff = column offset into mpack.
      mpack: [128, Wtot] float32 packed mask bounding boxes (S^T layout).
    """
    plan = []
    cols = []
    widths = 0
    for (q0, q1) in QCH:
        sub = mask_perm[q0:q1, :]
        items = []
        for kt in range(NKT):
            k0, k1 = kt * KT, min((kt + 1) * KT, T)
            m = sub[:, k0:k1]
            if not m.any():
                continue
            kw = k1 - k0
            if m.all():
                items.append((kt, kw, None, 0))
            else:
                mt = m.T  # [kw, nq]  S^T layout
                z = ~mt
                rr = np.nonzero(z.any(axis=1))[0]
                cc = np.nonzero(z.any(axis=0))[0]
                r0, r1 = int(rr[0]), int(rr[-1]) + 1
                # engine partition windows: start 0 (any count) or 64 (<=64)
                r0 = 0 if r0 < 64 else 64
                c0, c1 = int(cc[0]), int(cc[-1]) + 1
                tilefrag = np.ones((128, c1 - c0), np.float32)
                tilefrag[r0:r1, :] = mt[r0:r1, c0:c1].astype(np.float32)
                items.append((kt, kw, (r0, r1, c0, c1), widths))
                cols.append(tilefrag)
                widths += c1 - c0
        plan.append(tuple(items))
    if widths == 0:
        mpack = np.zeros((128, 4), np.float32)
    else:
        mpack = np.concatenate(cols, axis=1)
    return tuple(plan), np.ascontiguousarray(mpack)


def _plan_signature(plan, wtot):
    return (tuple(plan), wtot)


_BUILD_CACHE = {}


def _split_excess_waits(nc, max_waits=1):
    """walrus (this build) rejects instructions with >2 sem-wait commands.

    Tile's kernel-tail drain waits on every live semaphore in one Drain;
    split the excess onto preceding same-engine instructions (extra Drains
    for InstDrain, NoOps otherwise).
    """
    import copy

    for bb in nc.main_func.blocks:
        insts = bb.instructions
        i = 0
        while i < len(insts):
            ins = insts[i]
            si = ins.sync_info
            mw = max_waits
            if si is not None and len(si.on_wait) > mw:
                waits = list(si.on_wait)
                extra = waits[:-mw]
                newones = []
                for j in range(0, len(extra), max_waits):  # nops take 2
                    if ins.__class__.__name__ == "InstDrain":
                        d = mybir.InstDrain(
                            name=f"{ins.name}-sw{j}", engine=ins.engine
                        )
                    else:
                        d = mybir.InstNoOp(name=f"{ins.name}-sw{j}", engine=ins.engine)
                    si2 = copy.deepcopy(si)
                    si2.on_wait = extra[j:j + max_waits]
                    si2.on_update = []
                    d.sync_info = si2
                    newones.append(d)
                si.on_wait = waits[-mw:]
                for d in reversed(newones):
                    insts.insert(i, d)
                i += len(newones)
            i += 1


def _build(plan, wtot, split=True):
    key = (_plan_signature(plan, wtot), split)
    if key in _BUILD_CACHE:
        return _BUILD_CACHE[key]

    nc = bass.Bass()
    xT = nc.declare_dram_parameter("xT", [C, TP], F32R, isOutput=False)
    wa = nc.declare_dram_parameter("wa", [C, 3 * NHG * HD], F32R, isOutput=False)
    wp = nc.declare_dram_parameter("wp", [NHG * HD, C], F32R, isOutput=False)
    mp = nc.declare_dram_parameter("mp", [128, max(wtot, 4)], F32R, isOutput=False)
    out = nc.declare_dram_parameter("out", [T, C], F32, isOutput=True)

    with tile.TileContext(nc) as tc:
        with ExitStack() as ctx:
            const = ctx.enter_context(tc.tile_pool(name="const", bufs=1))

            wa_sb = []
            wp_sb = []
            for k in range(KC):
                t_ = const.tile([128, 3 * NHG * HD], F32R, tag=f"wa{k}", name=f"wa{k}")
                nc.sync.dma_start(out=t_[:, :], in_=wa[k * 128:(k + 1) * 128, :])
                wa_sb.append(t_)
            for k in range(3):
                t_ = const.tile([128, C], F32R, tag=f"wp{k}", name=f"wp{k}")
                nc.sync.dma_start(out=t_[:, :], in_=wp[k * 128:(k + 1) * 128, :])
                wp_sb.append(t_)

            # packed partial-mask bounding boxes, resident in SBUF
            mk_sb = {}
            for ci in range(len(QCH)):
                for (kt, kw, bbox, moff) in plan[ci]:
                    if bbox is None:
                        continue
                    r0, r1, c0, c1 = bbox
                    w = c1 - c0
                    t_ = const.tile([128, w], F32R, tag=f"mk{ci}_{kt}", name=f"mk{ci}_{kt}")
                    nc.sync.dma_start(
                        out=t_[r0:r1, :], in_=mp[r0:r1, moff:moff + w]
                    )
                    mk_sb[(ci, kt)] = t_

            mones = const.tile([2, 64], F32R, tag="mones", name="mones")
            nc.vector.memset(mones[:, :].bitcast(F32), -0.5)
            nc.vector.tensor_copy(mones[:, :], mones[:, :].bitcast(F32))

            qt_sb = [const.tile([128, TP], F32R, tag=f"qt{p}", name=f"qt{p}") for p in range(NPACK)]
            kt_sb = [const.tile([128, TP], F32R, tag=f"kt{p}", name=f"ktt{p}") for p in range(NPACK)]
            v6_sb = [const.tile([128, NHG * HD1], F32R, tag=f"v6{t}", name=f"v6{t}") for t in range(NKT)]
            yt_sb = [const.tile([128, TP], F32R, tag=f"yt{p}", name=f"yt{p}") for p in range(NPACK)]

            # ---------------- Phase B: qkv projections ----------------
            with tc.tile_pool(name="xtp", bufs=1) as xtp, \
                 tc.tile_pool(name="pb", bufs=3, space="PSUM") as pb, \
                 tc.tile_pool(name="pv", bufs=3, space="PSUM") as pvp:
                xt_sb = []
                for k in range(KC):
                    t_ = xtp.tile([128, TP], F32R, tag=f"xt{k}", name=f"xt{k}")
                    nc.sync.dma_start(out=t_[:, :], in_=xT[k * 128:(k + 1) * 128, :])
                    xt_sb.append(t_)

                # Q^T and K^T, packed 2 heads per 128-partition tile
                for p in range(NPACK):
                    for (q0, q1) in QCHC:
                        n = q1 - q0
                        for j, dst in ((0, qt_sb), (1, kt_sb)):
                            ps = pb.tile([128, 512], F32, tag="pb", name="pbt")
                            col = j * NHG * HD + p * 128
                            for k in range(KC):
                                nc.tensor.matmul(
                                    ps[:, 0:n],
                                    wa_sb[k][:, col:col + 128],
                                    xt_sb[k][:, q0:q1],
                                    start=(k == 0), stop=(k == KC - 1),
                                )
                            nc.vector.tensor_copy(dst[p][:, q0:q1], ps[:, 0:n])

                # V in natural [token, head*hd] layout, interleaved with ones col
                for t in range(NKT):
                    tw = min(128, T - t * KT)
                    twp = tw if tw % 2 == 0 else tw + 1
                    ps = pvp.tile([128, NHG * HD], F32, tag="pv", name="pvt")
                    for k in range(KC):
                        nc.tensor.matmul(
                            ps[0:twp, :],
                            xt_sb[k][:, t * KT:t * KT + twp],
                            wa_sb[k][:, 2 * NHG * HD:3 * NHG * HD],
                            start=(k == 0), stop=(k == KC - 1),
                        )
                    v6v = v6_sb[t].rearrange("a (h d) -> a h d", d=HD1)
                    nc.vector.memset(v6_sb[t][:, :].bitcast(F32), 0.0)
                    psv = ps.rearrange("a (h d) -> a h d", d=HD)
                    nc.vector.tensor_copy(v6v[0:tw, :, 0:HD], psv[0:tw, :, :])
                    ones_v = v6v[0:tw, :, HD:HD + 2]
                    nc.vector.memset(ones_v.bitcast(F32), 1.0)
                    nc.vector.tensor_copy(ones_v, ones_v.bitcast(F32))

            # ---------------- Phase C: attention ----------------
            with tc.tile_pool(name="sps", bufs=2, space="PSUM") as sps, \
                 tc.tile_pool(name="ups", bufs=2, space="PSUM") as ups, \
                 tc.tile_pool(name="rps", bufs=2, space="PSUM") as rps, \
                 tc.tile_pool(name="epool", bufs=3) as epool, \
                 tc.tile_pool(name="npool", bufs=4) as npool:
                mmalt = 0
                for p in range(NPACK):
                    for ci, (q0, q1) in enumerate(QCHC):
                        n = q1 - q0
                        items = plan[ci]
                        first_kt = items[0][0]
                        last_kt = items[-1][0]
                        u2 = [ups.tile([HD1, n], F32, tag="u", name="ut") for _ in (0, 1)]
                        for (kt, kw, bbox, _moff) in items:
                            kwp = kw if kw % 2 == 0 else kw + 1
                            st = sps.tile([128, 2, 512], F32, tag="s", name="st")
                            for e in (0, 1):
                                nc.tensor.matmul(
                                    st[0:kwp, e, 0:n],
                                    kt_sb[p][e * 64:(e + 1) * 64,
                                             kt * KT:kt * KT + kwp],
                                    qt_sb[p][e * 64:(e + 1) * 64, q0:q1],
                                    start=True, stop=True,
                                )
                            et = epool.tile([128, 2, 512], F32R, tag="e", name="et")
                            nc.scalar.activation(
                                et[0:kwp, :, 0:n], st[0:kwp, :, 0:n], AF.Exp, scale=0.125
                            )
                            if bbox is not None:
                                r0, r1, c0, c1 = bbox
                                mk = mk_sb[(ci, kt)]
                                for e in (0, 1):
                                    eng = nc.vector
                                    mmalt += 1
                                    eng.tensor_mul(
                                        et[r0:r1, e, c0:c1],
                                        et[r0:r1, e, c0:c1],
                                        mk[r0:r1, 0:c1 - c0],
                                    )
                            for e in (0, 1):
                                h = 2 * p + e
                                nc.tensor.matmul(
                                    u2[e][0:HD1, 0:n],
                                    v6_sb[kt][0:kwp, h * HD1:(h + 1) * HD1],
                                    et[0:kwp, e, 0:n],
                                    start=(kt == first_kt), stop=(kt == last_kt),
                                )
                        for e in (0, 1):
                            lnd = npool.tile([2, n], F32R, tag="lnd", name="lnd")
                            nc.scalar.activation(lnd[0:2, :], u2[e][64:66, 0:n], AF.Ln)
                            rb = rps.tile([64, n], F32, tag="rb", name="rbt")
                            nc.tensor.matmul(
                                rb[0:64, :],
                                mones[0:2, 0:64],
                                lnd[0:2, :],
                                start=True, stop=True,
                            )
                            rbs = npool.tile([64, n], F32, tag="rbs", name="rbs")
                            nc.scalar.activation(rbs[:, :], rb[0:64, :], AF.Exp)
                            nc.vector.tensor_mul(
                                yt_sb[p][e * 64:(e + 1) * 64, q0:q1],
                                u2[e][0:64, 0:n],
                                rbs[:, :],
                            )

            # ---------------- Phase D: output projection ----------------
            with tc.tile_pool(name="ops", bufs=4, space="PSUM") as ops, \
                 tc.tile_pool(name="osb", bufs=4) as osb:
                for t in range(NKT):
                    tw = min(128, T - t * KT)
                    twp = tw if tw % 2 == 0 else tw + 1
                    for (n0, n1) in ((0, 384), (384, 768)):
                        po = ops.tile([128, 384], F32, tag="o", name="ot_ps")
                        for k3 in range(3):
                            nc.tensor.matmul(
                                po[0:twp, :],
                                yt_sb[k3][:, t * KT:t * KT + twp],
                                wp_sb[k3][:, n0:n1],
                                start=(k3 == 0), stop=(k3 == 2),
                            )
                        ot = osb.tile([128, 384], F32, tag="ot", name="ot_sb")
                        nc.vector.tensor_copy(ot[0:tw, :], po[0:tw, :])
                        nc.sync.dma_start(
                            out=out[t * KT:t * KT + tw, n0:n1], in_=ot[0:tw, :]
                        )

    if split:
        _split_excess_waits(nc)
    _BUILD_CACHE[key] = nc
    return nc


def _prep_inputs(x, W_attn, W_proj, mpack):
    """Per-core input maps. core c -> batch c//2, head-group c%2."""
    x = np.asarray(x, np.float32)
    W_attn = np.asarray(W_attn, np.float32)
    W_proj = np.asarray(W_proj, np.float32)
    in_maps = []
    xT_by_batch = []
    for b in range(B):
        xt = np.zeros((C, TP), np.float32)
        xt[:, :T] = x[b][PERM, :].T
        xT_by_batch.append(xt)
    for c in range(NCORES):
        b, g = c // 2, c % 2
        cs = slice(g * NHG * HD, (g + 1) * NHG * HD)
        wa_s = np.ascontiguousarray(
            np.concatenate(
                [W_attn[:, cs], W_attn[:, C:][:, cs], W_attn[:, 2 * C:][:, cs]],
                axis=1,
            )
        )
        wp_s = np.ascontiguousarray(W_proj[cs, :])
        in_maps.append(
            {"xT": xT_by_batch[b], "wa": wa_s, "wp": wp_s, "mp": mpack}
        )
    return in_maps


def _run(inputs, trace=False, trace_cores=None):
    x = np.asarray(inputs["x"], np.float32)
    mask = np.asarray(inputs["mask"], bool)
    mask_perm = mask[np.ix_(PERM, PERM)]
    plan, mpack = _analyze(mask_perm)
    nc = _build(plan, mpack.shape[1])
    in_maps = _prep_inputs(x, inputs["W_attn"], inputs["W_proj"], mpack)
    res = run_bass_kernel_spmd(
        nc, in_maps, list(range(NCORES)), trace=trace, trace_cores=trace_cores
    )
    outs = [np.asarray(r["out"], np.float32) for r in res.results]
    full = np.empty((B, T, C), np.float32)
    for b in range(B):
        comb = outs[2 * b] + outs[2 * b + 1]
        full[b][PERM, :] = comb
    return full, res


def kernel(**inputs) -> np.ndarray:
    out, _ = _run(inputs)
    return out



# revision 3
# speedup vs baseline: 1.2271x; 1.2271x over previous
"""Block-sparse causal self-attention on 8 TRN2 NeuronCores (SPMD Bass/Tile kernel).

Sharding: core c -> (batch b = c//2, head-group g = c%2 of 6 heads).
Each core computes qkv projection (its 6 heads), masked attention, and a
partial output projection (its 384 rows of W_proj).  Host sums the two
partials per batch and concatenates batches.

Token reorder (host-side permutation, inverted on output):
  [U_0 .. U_7 | A]  with U_i = [tactile_i (16), image_i (196)], A = 9 actions.
This makes the block-sparse mask nearly block-lower-triangular with
frame-aligned boundaries, so most 128-wide key tiles are either fully
visible or fully masked; the few partial tiles get an elementwise
multiply restricted to the bounding box of their masked region.

Attention is computed in transposed layout S^T[k, q] so that softmax
normalization comes from ones-columns appended to V (rowsum lands in the
PV matmul output) and no on-chip transposes are needed anywhere.

v2: all matmul operands are bf16 (fp32 PSUM accumulation) - this cuts
LDWEIGHTS time ~3x vs fp32r and halves HBM traffic; S/PV/exp are trimmed
to the visible query span of each key tile; softmax normalization uses a
DVE reciprocal (no Ln/Exp activations); input DMAs are issued in
contraction-k order, x chunked, so the first projection matmul starts
within a few us of kernel start; output is written bf16 and upcast on
the host.
"""

import os
import sys
from contextlib import ExitStack

import numpy as np

for _p in ("/opt/trn_rl_repo", "/root/.axon_site/_ro/trn_rl_repo"):
    if os.path.isdir(_p) and _p not in sys.path:
        sys.path.insert(0, _p)

import concourse.bass as bass
import concourse.tile as tile
from concourse import mybir
from concourse.bass_utils import run_bass_kernel_spmd

F32 = mybir.dt.float32
BF16 = mybir.dt.bfloat16
NPBF16 = mybir.dt.np(BF16)
AF = mybir.ActivationFunctionType

L, PP, PT = 8, 196, 16
T, C, NH, B, HD = 1705, 768, 12, 4, 64
NCORES = 8
NHG = NH // 2          # heads per core = 6
NPACK = NHG // 2       # head pairs per core = 3
KC = C // 128          # 6 contraction tiles over C
KT = 128               # key tile size
NKT = (T + KT - 1) // KT   # 14
TP = 1706              # T padded to even
# frame-aligned query chunks in permuted order [U_0..U_7 | A]
QCH = [(0, 424), (424, 848), (848, 1272), (1272, T)]
QCHC = [(0, 424), (424, 848), (848, 1272), (1272, TP)]  # compute chunks (even n)
HD1 = HD + 2           # V width: 64 V cols | ones col | ones col


def _perm():
    idx = []
    for i in range(L):
        idx += list(range(9 + PT * i, 9 + PT * (i + 1)))
        idx += list(range(9 + L * PT + PP * i, 9 + L * PT + PP * (i + 1)))
    idx += list(range(0, 9))
    return np.asarray(idx, dtype=np.int64)


PERM = _perm()


def _analyze(mask_perm):
    """Compile-time plan from the (permuted) boolean mask.

    Returns (plan, mpack):
      plan: per query-chunk, list of (kt, kw, qv0, qv1, bbox-or-None, moff):
            qv0/qv1 = chunk-relative visible query span (even-aligned);
            bbox = (r0, r1, c0, c1) of the masked (zero) region inside the
            S^T tile [kw keys x chunk queries], clipped to the span;
            moff = column offset into mpack.
      mpack: [128, Wtot] float32 packed mask bounding boxes (S^T layout).
    """
    plan = []
    cols = []
    widths = 0
    for (q0, q1) in QCH:
        sub = mask_perm[q0:q1, :]
        nq = sub.shape[0]
        items = []
        for kt in range(NKT):
            k0, k1 = kt * KT, min((kt + 1) * KT, T)
            m = sub[:, k0:k1]
            if not m.any():
                continue
            kw = k1 - k0
            qv = np.nonzero(m.any(axis=1))[0]
            qv0 = int(qv[0]) & ~1
            qv1 = min(nq + (nq & 1), (int(qv[-1]) + 2) & ~1)
            if m.all():
                items.append((kt, kw, qv0, qv1, None, 0))
            else:
                mt = m.T  # [kw, nq]  S^T layout
                z = ~mt
                rr = np.nonzero(z.any(axis=1))[0]
                cc = np.nonzero(z.any(axis=0))[0]
                r0, r1 = int(rr[0]), int(rr[-1]) + 1
                # engine partition windows: start 0 (any count) or 64 (<=64)
                r0 = 0 if r0 < 64 else 64
                c0 = max(int(cc[0]), qv0)
                c1 = min(int(cc[-1]) + 1, qv1)
                if c0 >= c1:
                    items.append((kt, kw, qv0, qv1, None, 0))
                    continue
                tilefrag = np.ones((128, c1 - c0), np.float32)
                tilefrag[r0:r1, :] = mt[r0:r1, c0:c1].astype(np.float32)
                items.append((kt, kw, qv0, qv1, (r0, r1, c0, c1), widths))
                cols.append(tilefrag)
                widths += c1 - c0
        # PSUM accumulation relies on the first tile covering the full chunk
        assert items[0][2] == 0 and items[0][3] >= nq
        plan.append(tuple(items))
    if widths == 0:
        mpack = np.zeros((128, 4), np.float32)
    else:
        mpack = np.concatenate(cols, axis=1)
    return tuple(plan), np.ascontiguousarray(mpack)


_BUILD_CACHE = {}


def _split_excess_waits(nc, max_waits=1):
    """walrus (this build) rejects instructions with >2 sem-wait commands.

    Tile's kernel-tail drain waits on every live semaphore in one Drain;
    split the excess onto preceding same-engine instructions (extra Drains
    for InstDrain, NoOps otherwise).
    """
    import copy

    for bb in nc.main_func.blocks:
        insts = bb.instructions
        i = 0
        while i < len(insts):
            ins = insts[i]
            si = ins.sync_info
            mw = max_waits
            if si is not None and len(si.on_wait) > mw:
                waits = list(si.on_wait)
                extra = waits[:-mw]
                newones = []
                for j in range(0, len(extra), max_waits):  # nops take 2
                    if ins.__class__.__name__ == "InstDrain":
                        d = mybir.InstDrain(
                            name=f"{ins.name}-sw{j}", engine=ins.engine
                        )
                    else:
                        d = mybir.InstNoOp(name=f"{ins.name}-sw{j}", engine=ins.engine)
                    si2 = copy.deepcopy(si)
                    si2.on_wait = extra[j:j + max_waits]
                    si2.on_update = []
                    d.sync_info = si2
                    newones.append(d)
                si.on_wait = waits[-mw:]
                for d in reversed(newones):
                    insts.insert(i, d)
                i += len(newones)
            i += 1


def _build(plan, wtot, split=True):
    key = (tuple(plan), wtot, split)
    if key in _BUILD_CACHE:
        return _BUILD_CACHE[key]

    nc = bass.Bass()
    xT = nc.declare_dram_parameter("xT", [C, TP], BF16, isOutput=False)
    wa = nc.declare_dram_parameter("wa", [C, 3 * NHG * HD], BF16, isOutput=False)
    wp = nc.declare_dram_parameter("wp", [NHG * HD, C], BF16, isOutput=False)
    mp = nc.declare_dram_parameter("mp", [128, max(wtot, 4)], BF16, isOutput=False)
    out = nc.declare_dram_parameter("out", [T, C], BF16, isOutput=True)

    with tile.TileContext(nc) as tc:
        with ExitStack() as ctx:
            const = ctx.enter_context(tc.tile_pool(name="const", bufs=1))

            # ---- input DMAs in contraction-k order: wa[k] then xt[k] ----
            wa_sb = []
            xt_sb = []
            for k in range(KC):
                ta = const.tile([128, 3 * NHG * HD], BF16, tag=f"wa{k}", name=f"wa{k}")
                nc.sync.dma_start(out=ta[:, :], in_=wa[k * 128:(k + 1) * 128, :])
                tx = const.tile([128, TP], BF16, tag=f"xt{k}", name=f"xt{k}")
                for (q0, q1) in QCHC:
                    nc.sync.dma_start(
                        out=tx[:, q0:q1], in_=xT[k * 128:(k + 1) * 128, q0:q1]
                    )
                wa_sb.append(ta)
                xt_sb.append(tx)

            wp_sb = []
            for k in range(3):
                t_ = const.tile([128, C], BF16, tag=f"wp{k}", name=f"wp{k}")
                nc.sync.dma_start(out=t_[:, :], in_=wp[k * 128:(k + 1) * 128, :])
                wp_sb.append(t_)

            # packed partial-mask bounding boxes, resident in SBUF
            mk_sb = {}
            for ci in range(len(QCH)):
                for (kt, kw, qv0, qv1, bbox, moff) in plan[ci]:
                    if bbox is None:
                        continue
                    r0, r1, c0, c1 = bbox
                    w = c1 - c0
                    t_ = const.tile([128, w], BF16, tag=f"mk{ci}_{kt}", name=f"mk{ci}_{kt}")
                    nc.sync.dma_start(
                        out=t_[r0:r1, :], in_=mp[r0:r1, moff:moff + w]
                    )
                    mk_sb[(ci, kt)] = t_

            mones = const.tile([2, 64], BF16, tag="mones", name="mones")
            nc.vector.memset(mones[:, :], 0.5)

            qt_sb = [const.tile([128, TP], BF16, tag=f"qt{p}", name=f"qt{p}") for p in range(NPACK)]
            kt_sb = [const.tile([128, TP], BF16, tag=f"kt{p}", name=f"ktt{p}") for p in range(NPACK)]
            v6_sb = [const.tile([128, NHG * HD1], BF16, tag=f"v6{t}", name=f"v6{t}") for t in range(NKT)]
            yt_sb = [const.tile([128, TP], BF16, tag=f"yt{p}", name=f"yt{p}") for p in range(NPACK)]

            # ---------------- Phase B: qkv projections ----------------
            with tc.tile_pool(name="pb", bufs=3, space="PSUM") as pb, \
                 tc.tile_pool(name="pv", bufs=3, space="PSUM") as pvp:
                # Q^T and K^T, packed 2 heads per 128-partition tile
                for p in range(NPACK):
                    for (q0, q1) in QCHC:
                        n = q1 - q0
                        for j, dst in ((0, qt_sb), (1, kt_sb)):
                            ps = pb.tile([128, 512], F32, tag="pb", name="pbt")
                            col = j * NHG * HD + p * 128
                            for k in range(KC):
                                nc.tensor.matmul(
                                    ps[:, 0:n],
                                    wa_sb[k][:, col:col + 128],
                                    xt_sb[k][:, q0:q1],
                                    start=(k == 0), stop=(k == KC - 1),
                                )
                            nc.vector.tensor_copy(dst[p][:, q0:q1], ps[:, 0:n])

                # V in natural [token, head*hd] layout, interleaved with ones col
                for t in range(NKT):
                    tw = min(128, T - t * KT)
                    twp = tw if tw % 2 == 0 else tw + 1
                    ps = pvp.tile([128, NHG * HD], F32, tag="pv", name="pvt")
                    for k in range(KC):
                        nc.tensor.matmul(
                            ps[0:twp, :],
                            xt_sb[k][:, t * KT:t * KT + twp],
                            wa_sb[k][:, 2 * NHG * HD:3 * NHG * HD],
                            start=(k == 0), stop=(k == KC - 1),
                        )
                    v6v = v6_sb[t].rearrange("a (h d) -> a h d", d=HD1)
                    nc.vector.memset(v6_sb[t][:, :], 0.0)
                    psv = ps.rearrange("a (h d) -> a h d", d=HD)
                    nc.vector.tensor_copy(v6v[0:tw, :, 0:HD], psv[0:tw, :, :])
                    nc.vector.memset(v6v[0:tw, :, HD:HD + 2], 1.0)

            # ---------------- Phase C: attention ----------------
            with tc.tile_pool(name="sps", bufs=2, space="PSUM") as sps, \
                 tc.tile_pool(name="ups", bufs=2, space="PSUM") as ups, \
                 tc.tile_pool(name="rps", bufs=2, space="PSUM") as rps, \
                 tc.tile_pool(name="epool", bufs=3) as epool, \
                 tc.tile_pool(name="npool", bufs=4) as npool:
                for p in range(NPACK):
                    for ci, (q0, q1) in enumerate(QCHC):
                        n = q1 - q0
                        items = plan[ci]
                        first_kt = items[0][0]
                        last_kt = items[-1][0]
                        u2 = [ups.tile([HD1, n], F32, tag="u", name="ut") for _ in (0, 1)]
                        for (kt, kw, qv0, qv1, bbox, _moff) in items:
                            kwp = kw if kw % 2 == 0 else kw + 1
                            nv = qv1 - qv0
                            st = sps.tile([128, 2, 512], F32, tag="s", name="st")
                            for e in (0, 1):
                                nc.tensor.matmul(
                                    st[0:kwp, e, qv0:qv1],
                                    kt_sb[p][e * 64:(e + 1) * 64,
                                             kt * KT:kt * KT + kwp],
                                    qt_sb[p][e * 64:(e + 1) * 64,
                                             q0 + qv0:q0 + qv1],
                                    start=True, stop=True,
                                )
                            et = epool.tile([128, 2, 512], BF16, tag="e", name="et")
                            nc.scalar.activation(
                                et[0:kwp, :, qv0:qv1], st[0:kwp, :, qv0:qv1],
                                AF.Exp, scale=0.125,
                            )
                            if bbox is not None:
                                r0, r1, c0, c1 = bbox
                                mk = mk_sb[(ci, kt)]
                                for e in (0, 1):
                                    nc.vector.tensor_mul(
                                        et[r0:r1, e, c0:c1],
                                        et[r0:r1, e, c0:c1],
                                        mk[r0:r1, 0:c1 - c0],
                                    )
                            for e in (0, 1):
                                h = 2 * p + e
                                nc.tensor.matmul(
                                    u2[e][0:HD1, qv0:qv1],
                                    v6_sb[kt][0:kwp, h * HD1:(h + 1) * HD1],
                                    et[0:kwp, e, qv0:qv1],
                                    start=(kt == first_kt), stop=(kt == last_kt),
                                    skip_group_check=(kt != first_kt),
                                )
                        for e in (0, 1):
                            rcf = npool.tile([2, n], F32, tag="rcf", name="rcf")
                            nc.vector.reciprocal(rcf[0:2, :], u2[e][64:66, 0:n])
                            rcb = npool.tile([2, n], BF16, tag="rcb", name="rcb")
                            nc.vector.tensor_copy(rcb[0:2, :], rcf[0:2, :])
                            rb = rps.tile([64, n], F32, tag="rb", name="rbt")
                            nc.tensor.matmul(
                                rb[0:64, :],
                                mones[0:2, 0:64],
                                rcb[0:2, :],
                                start=True, stop=True,
                            )
                            rbs = npool.tile([64, n], BF16, tag="rbs", name="rbs")
                            nc.vector.tensor_copy(rbs[:, :], rb[0:64, :])
                            nc.vector.tensor_mul(
                                yt_sb[p][e * 64:(e + 1) * 64, q0:q1],
                                u2[e][0:64, 0:n],
                                rbs[0:64, 0:n],
                            )

            # ---------------- Phase D: output projection ----------------
            with tc.tile_pool(name="ops", bufs=4, space="PSUM") as ops, \
                 tc.tile_pool(name="osb", bufs=4) as osb:
                for t in range(NKT):
                    tw = min(128, T - t * KT)
                    twp = tw if tw % 2 == 0 else tw + 1
                    for (n0, n1) in ((0, 384), (384, 768)):
                        po = ops.tile([128, 384], F32, tag="o", name="ot_ps")
                        for k3 in range(3):
                            nc.tensor.matmul(
                                po[0:twp, :],
                                yt_sb[k3][:, t * KT:t * KT + twp],
                                wp_sb[k3][:, n0:n1],
                                start=(k3 == 0), stop=(k3 == 2),
                            )
                        ot = osb.tile([128, 384], BF16, tag="ot", name="ot_sb")
                        nc.vector.tensor_copy(ot[0:tw, :], po[0:tw, :])
                        nc.sync.dma_start(
                            out=out[t * KT:t * KT + tw, n0:n1], in_=ot[0:tw, :]
                        )

    if split:
        _split_excess_waits(nc)
    _BUILD_CACHE[key] = nc
    return nc


def _prep_inputs(x, W_attn, W_proj, mpack):
    """Per-core input maps. core c -> batch c//2, head-group c%2."""
    x = np.asarray(x, np.float32)
    W_attn = np.asarray(W_attn, np.float32)
    W_proj = np.asarray(W_proj, np.float32)
    mpack_bf = mpack.astype(NPBF16)
    in_maps = []
    xT_by_batch = []
    for b in range(B):
        xt = np.zeros((C, TP), NPBF16)
        xt[:, :T] = x[b][PERM, :].T.astype(NPBF16)
        xT_by_batch.append(xt)
    for c in range(NCORES):
        b, g = c // 2, c % 2
        cs = slice(g * NHG * HD, (g + 1) * NHG * HD)
        wa_s = np.ascontiguousarray(
            np.concatenate(
                [W_attn[:, cs], W_attn[:, C:][:, cs], W_attn[:, 2 * C:][:, cs]],
                axis=1,
            ).astype(NPBF16)
        )
        wp_s = np.ascontiguousarray(W_proj[cs, :].astype(NPBF16))
        in_maps.append(
            {"xT": xT_by_batch[b], "wa": wa_s, "wp": wp_s, "mp": mpack_bf}
        )
    return in_maps


def _run(inputs, trace=False, trace_cores=None):
    x = np.asarray(inputs["x"], np.float32)
    mask = np.asarray(inputs["mask"], bool)
    mask_perm = mask[np.ix_(PERM, PERM)]
    plan, mpack = _analyze(mask_perm)
    nc = _build(plan, mpack.shape[1])
    in_maps = _prep_inputs(x, inputs["W_attn"], inputs["W_proj"], mpack)
    res = run_bass_kernel_spmd(
        nc, in_maps, list(range(NCORES)), trace=trace, trace_cores=trace_cores
    )
    outs = [np.asarray(r["out"]).astype(np.float32) for r in res.results]
    full = np.empty((B, T, C), np.float32)
    for b in range(B):
        comb = outs[2 * b] + outs[2 * b + 1]
        full[b][PERM, :] = comb
    return full, res


def kernel(**inputs) -> np.ndarray:
    out, _ = _run(inputs)
    return out


# revision 11
# speedup vs baseline: 1.2699x; 1.0349x over previous
"""Block-sparse causal self-attention on 8 TRN2 NeuronCores (SPMD Bass/Tile kernel).

Sharding: core c -> (batch b = c//2, head-group g = c%2 of 6 heads).
Each core computes qkv projection (its 6 heads), masked attention, and a
partial output projection (its 384 rows of W_proj).  Host sums the two
partials per batch and concatenates batches.

Token reorder (host-side permutation, inverted on output):
  [U_0 .. U_7 | A]  with U_i = [tactile_i (16), image_i (196)], A = 9 actions.
This makes the block-sparse mask nearly block-lower-triangular with
frame-aligned boundaries; the few partial tiles get an elementwise
multiply restricted to the bounding box of their masked region.

Attention is computed in transposed layout S^T[k, q]; softmax
normalization comes from ones-columns appended to V (rowsum lands in the
PV matmul output), a fast DVE reciprocal, and a tiny ones-matmul that
broadcasts 1/rowsum across 64 partitions.

v3 structure (all matmul operands bf16, fp32 PSUM):
 - program order interleaves projection / attention / output phases per
   query chunk so the ACT (exp) and DVE engines start within ~10us and
   no phase serializes the whole kernel;
 - within a chunk the S->exp->PV chain is software-pipelined: S(kt+1)
   issues between exp(kt) and PV(kt) so the tensor engine never waits
   on the activation engine;
 - softmax normalization is deferred by one chunk so its cross-engine
   chain never blocks the in-order tensor queue;
 - action-token K columns and V tile 13 are computed up front (the
   permutation puts action keys last, and every chunk attends to them);
 - one shared 2-buffer PSUM ring (4 banks) serves S tiles, projection
   groups and output-projection groups; u2 accumulators and the
   broadcast tile use the remaining 4 banks.
"""

import os
import sys
from contextlib import ExitStack

import numpy as np

for _p in ("/opt/trn_rl_repo", "/root/.axon_site/_ro/trn_rl_repo"):
    if os.path.isdir(_p) and _p not in sys.path:
        sys.path.insert(0, _p)

import concourse.bass as bass
import concourse.tile as tile
from concourse import mybir
from concourse.bass_utils import run_bass_kernel_spmd

F32 = mybir.dt.float32
BF16 = mybir.dt.bfloat16
NPBF16 = mybir.dt.np(BF16)
AF = mybir.ActivationFunctionType

L, PP, PT = 8, 196, 16
T, C, NH, B, HD = 1705, 768, 12, 4, 64
NCORES = 8
NHG = NH // 2          # heads per core = 6
NPACK = NHG // 2       # head pairs per core = 3
KC = C // 128          # 6 contraction tiles over C
KT = 128               # key tile size
NKT = (T + KT - 1) // KT   # 14
TP = 1706              # T padded to even
QCH = [(0, 424), (424, 848), (848, 1272), (1272, T)]
QCHC = [(0, 424), (424, 848), (848, 1272), (1272, TP)]  # compute chunks (even n)
HD1 = HD + 2           # V width: 64 V cols | ones col | ones col
KTAIL = 1664           # action-key tail start (tile 13), computed up front
QKB = [(0, 854), (854, 1706)]  # projection chunks


def _perm():
    idx = []
    for i in range(L):
        idx += list(range(9 + PT * i, 9 + PT * (i + 1)))
        idx += list(range(9 + L * PT + PP * i, 9 + L * PT + PP * (i + 1)))
    idx += list(range(0, 9))
    return np.asarray(idx, dtype=np.int64)


PERM = _perm()


def _analyze(mask_perm):
    """Compile-time plan from the (permuted) boolean mask.

    Returns (plan, mpack):
      plan: per query-chunk, tuple of (kt, kw, qv0, qv1, bbox-or-None, moff):
            qv0/qv1 = chunk-relative visible query span (even-aligned);
            bbox = (r0, r1, c0, c1) of the masked (zero) region inside the
            S^T tile [kw keys x chunk queries], clipped to the span;
            moff = column offset into mpack.
      mpack: [128, Wtot] float32 packed mask bounding boxes (S^T layout).
    """
    plan = []
    cols = []
    widths = 0
    for (q0, q1) in QCH:
        sub = mask_perm[q0:q1, :]
        nq = sub.shape[0]
        items = []
        for kt in range(NKT):
            k0, k1 = kt * KT, min((kt + 1) * KT, T)
            m = sub[:, k0:k1]
            if not m.any():
                continue
            kw = k1 - k0
            qv = np.nonzero(m.any(axis=1))[0]
            qv0 = int(qv[0]) & ~1
            qv1 = min(nq + (nq & 1), (int(qv[-1]) + 2) & ~1)
            if m.all():
                items.append((kt, kw, qv0, qv1, None, 0))
            else:
                mt = m.T  # [kw, nq]  S^T layout
                z = ~mt
                rr = np.nonzero(z.any(axis=1))[0]
                cc = np.nonzero(z.any(axis=0))[0]
                r0, r1 = int(rr[0]), int(rr[-1]) + 1
                # engine partition windows: start 0 (any count) or 64 (<=64)
                r0 = 0 if r0 < 64 else 64
                c0 = max(int(cc[0]), qv0)
                c1 = min(int(cc[-1]) + 1, qv1)
                if c0 >= c1:
                    items.append((kt, kw, qv0, qv1, None, 0))
                    continue
                tilefrag = np.ones((128, c1 - c0), np.float32)
                tilefrag[r0:r1, :] = mt[r0:r1, c0:c1].astype(np.float32)
                items.append((kt, kw, qv0, qv1, (r0, r1, c0, c1), widths))
                cols.append(tilefrag)
                widths += c1 - c0
        # PSUM accumulation relies on the first tile covering the full chunk
        assert items[0][2] == 0 and items[0][3] >= nq
        plan.append(tuple(items))
    if widths == 0:
        mpack = np.zeros((128, 4), np.float32)
    else:
        mpack = np.concatenate(cols, axis=1)
    return tuple(plan), np.ascontiguousarray(mpack)


_BUILD_CACHE = {}


def _split_excess_waits(nc, max_waits=1):
    """walrus (this build) rejects instructions with >2 sem-wait commands.

    Tile's kernel-tail drain waits on every live semaphore in one Drain;
    split the excess onto preceding same-engine instructions (extra Drains
    for InstDrain, NoOps otherwise).
    """
    import copy

    for bb in nc.main_func.blocks:
        insts = bb.instructions
        i = 0
        while i < len(insts):
            ins = insts[i]
            si = ins.sync_info
            mw = max_waits
            if si is not None and len(si.on_wait) > mw:
                waits = list(si.on_wait)
                extra = waits[:-mw]
                newones = []
                for j in range(0, len(extra), max_waits):  # nops take 2
                    if ins.__class__.__name__ == "InstDrain":
                        d = mybir.InstDrain(
                            name=f"{ins.name}-sw{j}", engine=ins.engine
                        )
                    else:
                        d = mybir.InstNoOp(name=f"{ins.name}-sw{j}", engine=ins.engine)
                    si2 = copy.deepcopy(si)
                    si2.on_wait = extra[j:j + max_waits]
                    si2.on_update = []
                    d.sync_info = si2
                    newones.append(d)
                si.on_wait = waits[-mw:]
                for d in reversed(newones):
                    insts.insert(i, d)
                i += len(newones)
            i += 1


def _build(plan, wtot, split=True):
    key = (tuple(plan), wtot, split)
    if key in _BUILD_CACHE:
        return _BUILD_CACHE[key]

    nc = bass.Bass()
    xT = nc.declare_dram_parameter("xT", [C, TP], BF16, isOutput=False)
    wa = nc.declare_dram_parameter("wa", [C, 3 * NHG * HD], BF16, isOutput=False)
    wp = nc.declare_dram_parameter("wp", [NHG * HD, C], BF16, isOutput=False)
    mp = nc.declare_dram_parameter("mp", [128, max(wtot, 4)], BF16, isOutput=False)
    out = nc.declare_dram_parameter("out", [T, C], BF16, isOutput=True)

    with tile.TileContext(nc) as tc:
        with ExitStack() as ctx:
            const = ctx.enter_context(tc.tile_pool(name="const", bufs=1))

            # ---- input DMAs in contraction-k order: wa[k] then xt[k] ----
            wa_sb = []
            xt_sb = []
            for k in range(KC):
                ta = const.tile([128, 3 * NHG * HD], BF16, tag=f"wa{k}", name=f"wa{k}")
                nc.sync.dma_start(out=ta[:, :], in_=wa[k * 128:(k + 1) * 128, :])
                tx = const.tile([128, TP], BF16, tag=f"xt{k}", name=f"xt{k}")
                for (q0, q1) in QCHC:
                    nc.sync.dma_start(
                        out=tx[:, q0:q1], in_=xT[k * 128:(k + 1) * 128, q0:q1]
                    )
                wa_sb.append(ta)
                xt_sb.append(tx)

            wp_sb = []
            for k in range(3):
                t_ = const.tile([128, C], BF16, tag=f"wp{k}", name=f"wp{k}")
                nc.sync.dma_start(out=t_[:, :], in_=wp[k * 128:(k + 1) * 128, :])
                wp_sb.append(t_)

            mk_sb = {}
            for ci in range(len(QCH)):
                for (kt, kw, qv0, qv1, bbox, moff) in plan[ci]:
                    if bbox is None:
                        continue
                    r0, r1, c0, c1 = bbox
                    w = c1 - c0
                    t_ = const.tile([128, w], BF16, tag=f"mk{ci}_{kt}", name=f"mk{ci}_{kt}")
                    nc.sync.dma_start(
                        out=t_[r0:r1, :], in_=mp[r0:r1, moff:moff + w]
                    )
                    mk_sb[(ci, kt)] = t_

            mones = const.tile([2, 64], BF16, tag="mones", name="mones")
            nc.vector.memset(mones[:, :], 0.5)

            qt_sb = [const.tile([128, TP], BF16, tag=f"qt{p}", name=f"qt{p}") for p in range(NPACK)]
            kt_sb = [const.tile([128, TP], BF16, tag=f"kt{p}", name=f"ktt{p}") for p in range(NPACK)]
            v6_sb = [const.tile([128, NHG * HD1], BF16, tag=f"v6{t}", name=f"v6{t}") for t in range(NKT)]
            yt_sb = [const.tile([128, TP], BF16, tag=f"yt{p}", name=f"yt{p}") for p in range(NPACK)]

            with tc.tile_pool(name="ps8", bufs=2, space="PSUM") as ps8, \
                 tc.tile_pool(name="epool", bufs=3) as epool, \
                 tc.tile_pool(name="ubp", bufs=14) as ubp, \
                 tc.tile_pool(name="npool", bufs=4) as npool, \
                 tc.tile_pool(name="osb", bufs=3) as osb:

                def big_ps():
                    # shared 2-buffer ring of 4KB (2-bank) PSUM tiles
                    return ps8.tile([128, 2, 512], F32, tag="s", name="st")

                # ---- phase emitters ------------------------------------
                def emit_qk_chunks(cis):
                    # PSUM matmul outputs must stay within one 2KB bank,
                    # so emit per QCHC sub-chunk (<=512 fp32 wide)
                    for p in range(NPACK):
                        for j, dst in ((0, qt_sb), (1, kt_sb)):
                            for ci in cis:
                                a, b = QCHC[ci]
                                k1 = b
                                if j == 1 and b > KTAIL:
                                    k1 = KTAIL  # tail K cols computed separately
                                nn = k1 - a
                                ps = big_ps().rearrange("a b c -> a (b c)")
                                col = j * NHG * HD + p * 128
                                for k in range(KC):
                                    nc.tensor.matmul(
                                        ps[:, 0:nn],
                                        wa_sb[k][:, col:col + 128],
                                        xt_sb[k][:, a:k1],
                                        start=(k == 0), stop=(k == KC - 1),
                                    )
                                nc.vector.tensor_copy(dst[p][:, a:k1], ps[:, 0:nn])

                def emit_k_tail():
                    n = TP - KTAIL  # 42
                    for p in range(NPACK):
                        ps = big_ps().rearrange("a b c -> a (b c)")
                        col = NHG * HD + p * 128
                        for k in range(KC):
                            nc.tensor.matmul(
                                ps[:, 0:n],
                                wa_sb[k][:, col:col + 128],
                                xt_sb[k][:, KTAIL:TP],
                                start=(k == 0), stop=(k == KC - 1),
                            )
                        nc.vector.tensor_copy(kt_sb[p][:, KTAIL:TP], ps[:, 0:n])

                def emit_v_tiles(ts):
                    for t in ts:
                        tw = min(128, T - t * KT)
                        twp = tw if tw % 2 == 0 else tw + 1
                        ps = big_ps().rearrange("a b c -> a (b c)")
                        for k in range(KC):
                            nc.tensor.matmul(
                                ps[0:twp, 0:NHG * HD],
                                xt_sb[k][:, t * KT:t * KT + twp],
                                wa_sb[k][:, 2 * NHG * HD:3 * NHG * HD],
                                start=(k == 0), stop=(k == KC - 1),
                            )
                        v6v = v6_sb[t].rearrange("a (h d) -> a h d", d=HD1)
                        if tw < 128:
                            nc.vector.memset(v6_sb[t][:, :], 0.0)
                        psv = ps[:, 0:NHG * HD].rearrange("a (h d) -> a h d", d=HD)
                        nc.vector.tensor_copy(v6v[0:tw, :, 0:HD], psv[0:tw, :, :])
                        nc.vector.memset(v6v[0:tw, :, HD:HD + 2], 1.0)

                pending = []   # deferred normalization closures

                def emit_attn_chunk(ci):
                    q0, q1 = QCHC[ci]
                    n = q1 - q0
                    items = plan[ci]
                    first_kt = items[0][0]
                    last_kt = items[-1][0]
                    for p in range(NPACK):
                        u2 = [ps8.tile([HD1, 448], F32, tag="u", name="ut")
                              for _ in (0, 1)]
                        sts = {}
                        ets = {}

                        def emit_s(idx):
                            kt, kw, qv0, qv1, bbox, _m = items[idx]
                            kwp = kw if kw % 2 == 0 else kw + 1
                            st = big_ps()
                            for e in (0, 1):
                                nc.tensor.matmul(
                                    st[0:kwp, e, qv0:qv1],
                                    kt_sb[p][e * 64:(e + 1) * 64,
                                             kt * KT:kt * KT + kwp],
                                    qt_sb[p][e * 64:(e + 1) * 64,
                                             q0 + qv0:q0 + qv1],
                                    start=True, stop=True,
                                )
                            sts[idx] = st

                        def emit_exp(idx):
                            kt, kw, qv0, qv1, bbox, _m = items[idx]
                            kwp = kw if kw % 2 == 0 else kw + 1
                            et = epool.tile([128, 2, 512], BF16, tag="e", name="et")
                            nc.scalar.activation(
                                et[0:kwp, :, qv0:qv1], sts[idx][0:kwp, :, qv0:qv1],
                                AF.Exp, scale=0.125,
                            )
                            ets[idx] = et

                        def emit_mask_pv(idx):
                            kt, kw, qv0, qv1, bbox, _m = items[idx]
                            kwp = kw if kw % 2 == 0 else kw + 1
                            et = ets.pop(idx)
                            sts.pop(idx)
                            if bbox is not None:
                                r0, r1, c0, c1 = bbox
                                mk = mk_sb[(ci, kt)]
                                for e in (0, 1):
                                    nc.vector.tensor_mul(
                                        et[r0:r1, e, c0:c1],
                                        et[r0:r1, e, c0:c1],
                                        mk[r0:r1, 0:c1 - c0],
                                    )
                            for e in (0, 1):
                                h = 2 * p + e
                                nc.tensor.matmul(
                                    u2[e][0:HD1, qv0:qv1],
                                    v6_sb[kt][0:kwp, h * HD1:(h + 1) * HD1],
                                    et[0:kwp, e, qv0:qv1],
                                    start=(kt == first_kt), stop=(kt == last_kt),
                                    skip_group_check=(kt != first_kt),
                                )

                        # software-pipelined S -> exp -> PV
                        emit_s(0)
                        for i in range(len(items)):
                            emit_exp(i)
                            if i + 1 < len(items):
                                emit_s(i + 1)
                            emit_mask_pv(i)

                        # drain u2 quickly; defer the normalization chain
                        ubs = []
                        rcs = []
                        for e in (0, 1):
                            ub = ubp.tile([HD1, 448], BF16, tag="ub", name="ub")
                            nc.vector.tensor_copy(ub[0:HD1, 0:n], u2[e][0:HD1, 0:n])
                            rcf = ubp.tile([2, 448], F32, tag="rcf", name="rcf")
                            nc.vector.reciprocal(rcf[0:2, 0:n], u2[e][64:66, 0:n])
                            ubs.append(ub)
                            rcs.append(rcf)

                        def norm(p=p, ci=ci, ubs=ubs, rcs=rcs, n=n, q0=q0, q1=q1):
                            for e in (0, 1):
                                rcb = npool.tile([2, 448], BF16, tag="rcb", name="rcb")
                                nc.vector.tensor_copy(rcb[0:2, 0:n], rcs[e][0:2, 0:n])
                                rb = ps8.tile([64, 448], F32, tag="rb", name="rbt")
                                nc.tensor.matmul(
                                    rb[0:64, 0:n],
                                    mones[0:2, 0:64],
                                    rcb[0:2, 0:n],
                                    start=True, stop=True,
                                )
                                nc.vector.tensor_mul(
                                    yt_sb[p][e * 64:(e + 1) * 64, q0:q1],
                                    ubs[e][0:64, 0:n],
                                    rb[0:64, 0:n],
                                )

                        pending.append(norm)

                def emit_norms():
                    while pending:
                        pending.pop(0)()

                def emit_d_tiles(ts):
                    for t in ts:
                        tw = min(128, T - t * KT)
                        twp = tw if tw % 2 == 0 else tw + 1
                        po = big_ps()  # two 384-wide halves, one per bank
                        for hb, (n0, n1) in enumerate(((0, 384), (384, 768))):
                            for k3 in range(3):
                                nc.tensor.matmul(
                                    po[0:twp, hb, 0:384],
                                    yt_sb[k3][:, t * KT:t * KT + twp],
                                    wp_sb[k3][:, n0:n1],
                                    start=(k3 == 0), stop=(k3 == 2),
                                )
                        ot = osb.tile([128, C], BF16, tag="ot", name="ot_sb")
                        nc.vector.tensor_copy(
                            ot.rearrange("a (b c) -> a b c", c=384)[0:tw, :, :],
                            po[0:tw, :, 0:384],
                        )
                        nc.sync.dma_start(
                            out=out[t * KT:t * KT + tw, :], in_=ot[0:tw, :]
                        )

                # ---- schedule ------------------------------------------
                # D tile t needs yt chunks up to (128*(t+1)-1)//424
                d_of = {}
                for t in range(NKT):
                    d_of.setdefault(min(3, (128 * (t + 1) - 1) // 424), []).append(t)

                emit_qk_chunks([0, 1])
                emit_k_tail()
                emit_v_tiles([13, 0, 1, 2, 3])
                emit_attn_chunk(0)
                emit_qk_chunks([2, 3])
                emit_v_tiles(range(4, 13))
                emit_attn_chunk(1)
                emit_norms()          # norms for chunks 0,1
                emit_attn_chunk(2)
                emit_d_tiles(d_of[0])
                emit_d_tiles(d_of[1])
                emit_attn_chunk(3)
                emit_norms()          # norms for chunks 2,3
                emit_d_tiles(d_of[2])
                emit_d_tiles(d_of[3])

    if split:
        _split_excess_waits(nc)
    _BUILD_CACHE[key] = nc
    return nc


def _prep_inputs(x, W_attn, W_proj, mpack):
    """Per-core input maps. core c -> batch c//2, head-group c%2."""
    x = np.asarray(x, np.float32)
    W_attn = np.asarray(W_attn, np.float32)
    W_proj = np.asarray(W_proj, np.float32)
    mpack_bf = mpack.astype(NPBF16)
    in_maps = []
    xT_by_batch = []
    for b in range(B):
        xt = np.zeros((C, TP), NPBF16)
        xt[:, :T] = x[b][PERM, :].T.astype(NPBF16)
        xT_by_batch.append(xt)
    for c in range(NCORES):
        b, g = c // 2, c % 2
        cs = slice(g * NHG * HD, (g + 1) * NHG * HD)
        wa_s = np.ascontiguousarray(
            np.concatenate(
                [W_attn[:, cs], W_attn[:, C:][:, cs], W_attn[:, 2 * C:][:, cs]],
                axis=1,
            ).astype(NPBF16)
        )
        wp_s = np.ascontiguousarray(W_proj[cs, :].astype(NPBF16))
        in_maps.append(
            {"xT": xT_by_batch[b], "wa": wa_s, "wp": wp_s, "mp": mpack_bf}
        )
    return in_maps


def _run(inputs, trace=False, trace_cores=None):
    x = np.asarray(inputs["x"], np.float32)
    mask = np.asarray(inputs["mask"], bool)
    mask_perm = mask[np.ix_(PERM, PERM)]
    plan, mpack = _analyze(mask_perm)
    nc = _build(plan, mpack.shape[1])
    in_maps = _prep_inputs(x, inputs["W_attn"], inputs["W_proj"], mpack)
    res = run_bass_kernel_spmd(
        nc, in_maps, list(range(NCORES)), trace=trace, trace_cores=trace_cores
    )
    outs = [np.asarray(r["out"]).astype(np.float32) for r in res.results]
    full = np.empty((B, T, C), np.float32)
    for b in range(B):
        comb = outs[2 * b] + outs[2 * b + 1]
        full[b][PERM, :] = comb
    return full, res


def kernel(**inputs) -> np.ndarray:
    out, _ = _run(inputs)
    return out


# revision 15
# speedup vs baseline: 1.4369x; 1.1315x over previous
"""Block-sparse causal self-attention on 8 TRN2 NeuronCores (SPMD Bass/Tile kernel).

Sharding: core c -> (batch b = c//2, head-group g = c%2 of 6 heads).
Each core computes qkv projection (its 6 heads), masked attention, and a
partial output projection (its 384 rows of W_proj).  Host sums the two
partials per batch and concatenates batches.

Token reorder (host-side permutation, inverted on output):
  [U_0 .. U_7 | A]  with U_i = [tactile_i (16), image_i (196)], A = 9 actions.
This makes the block-sparse mask nearly block-lower-triangular with
frame-aligned boundaries; the few partial tiles get an elementwise
multiply restricted to the bounding box of their masked region.

Attention is computed in transposed layout S^T[k, q]; softmax
normalization comes from ones-columns appended to V (rowsum lands in the
PV matmul output), a fast DVE reciprocal, and a tiny ones-matmul that
broadcasts 1/rowsum across 64 partitions.

v3 structure (all matmul operands bf16, fp32 PSUM):
 - program order interleaves projection / attention / output phases per
   query chunk so the ACT (exp) and DVE engines start within ~10us and
   no phase serializes the whole kernel;
 - within a chunk the S->exp->PV chain is software-pipelined: S(kt+1)
   issues between exp(kt) and PV(kt) so the tensor engine never waits
   on the activation engine;
 - softmax normalization is deferred by one chunk so its cross-engine
   chain never blocks the in-order tensor queue;
 - action-token K columns and V tile 13 are computed up front (the
   permutation puts action keys last, and every chunk attends to them);
 - one shared 2-buffer PSUM ring (4 banks) serves S tiles, projection
   groups and output-projection groups; u2 accumulators and the
   broadcast tile use the remaining 4 banks.
"""

import os
import sys
from contextlib import ExitStack

import numpy as np

for _p in ("/opt/trn_rl_repo", "/root/.axon_site/_ro/trn_rl_repo"):
    if os.path.isdir(_p) and _p not in sys.path:
        sys.path.insert(0, _p)

import concourse.bass as bass
import concourse.tile as tile
from concourse import mybir
from concourse.bass_utils import run_bass_kernel_spmd

F32 = mybir.dt.float32
BF16 = mybir.dt.bfloat16
NPBF16 = mybir.dt.np(BF16)
AF = mybir.ActivationFunctionType

L, PP, PT = 8, 196, 16
T, C, NH, B, HD = 1705, 768, 12, 4, 64
NCORES = 8
NHG = NH // 2          # heads per core = 6
NPACK = NHG // 2       # head pairs per core = 3
KC = C // 128          # 6 contraction tiles over C
KT = 128               # key tile size
NKT = (T + KT - 1) // KT   # 14
TP = 1706              # T padded to even
QCH = [(0, 424), (424, 848), (848, 1272), (1272, T)]
QCHC = [(0, 424), (424, 848), (848, 1272), (1272, TP)]  # compute chunks (even n)
HD1 = HD + 2           # V width: 64 V cols | ones col | ones col
KTAIL = 1664           # action-key tail start (tile 13), computed up front
QKB = [(0, 854), (854, 1706)]  # projection chunks


def _perm():
    idx = []
    for i in range(L):
        idx += list(range(9 + PT * i, 9 + PT * (i + 1)))
        idx += list(range(9 + L * PT + PP * i, 9 + L * PT + PP * (i + 1)))
    idx += list(range(0, 9))
    return np.asarray(idx, dtype=np.int64)


PERM = _perm()


def _analyze(mask_perm):
    """Compile-time plan from the (permuted) boolean mask.

    Returns (plan, mpack):
      plan: per query-chunk, tuple of (kt, kw, qv0, qv1, bbox-or-None, moff):
            qv0/qv1 = chunk-relative visible query span (even-aligned);
            bbox = (r0, r1, c0, c1) of the masked (zero) region inside the
            S^T tile [kw keys x chunk queries], clipped to the span;
            moff = column offset into mpack.
      mpack: [128, Wtot] float32 packed mask bounding boxes (S^T layout).
    """
    plan = []
    cols = []
    widths = 0
    for (q0, q1) in QCH:
        sub = mask_perm[q0:q1, :]
        nq = sub.shape[0]
        items = []
        for kt in range(NKT):
            k0, k1 = kt * KT, min((kt + 1) * KT, T)
            m = sub[:, k0:k1]
            if not m.any():
                continue
            kw = k1 - k0
            qv = np.nonzero(m.any(axis=1))[0]
            qv0 = int(qv[0]) & ~1
            qv1 = min(nq + (nq & 1), (int(qv[-1]) + 2) & ~1)
            if m.all():
                items.append((kt, kw, qv0, qv1, None, 0))
            else:
                mt = m.T  # [kw, nq]  S^T layout
                z = ~mt
                rr = np.nonzero(z.any(axis=1))[0]
                cc = np.nonzero(z.any(axis=0))[0]
                r0, r1 = int(rr[0]), int(rr[-1]) + 1
                # engine partition windows: start 0 (any count) or 64 (<=64)
                r0 = 0 if r0 < 64 else 64
                c0 = max(int(cc[0]), qv0)
                c1 = min(int(cc[-1]) + 1, qv1)
                if c0 >= c1:
                    items.append((kt, kw, qv0, qv1, None, 0))
                    continue
                tilefrag = np.ones((128, c1 - c0), np.float32)
                tilefrag[r0:r1, :] = mt[r0:r1, c0:c1].astype(np.float32)
                items.append((kt, kw, qv0, qv1, (r0, r1, c0, c1), widths))
                cols.append(tilefrag)
                widths += c1 - c0
        # PSUM accumulation relies on the first tile covering the full chunk
        assert items[0][2] == 0 and items[0][3] >= nq
        plan.append(tuple(items))
    if widths == 0:
        mpack = np.zeros((128, 4), np.float32)
    else:
        mpack = np.concatenate(cols, axis=1)
    return tuple(plan), np.ascontiguousarray(mpack)


_BUILD_CACHE = {}


def _split_excess_waits(nc, max_waits=1):
    """walrus (this build) rejects instructions with >2 sem-wait commands.

    Tile's kernel-tail drain waits on every live semaphore in one Drain;
    split the excess onto preceding same-engine instructions (extra Drains
    for InstDrain, NoOps otherwise).
    """
    import copy

    for bb in nc.main_func.blocks:
        insts = bb.instructions
        i = 0
        while i < len(insts):
            ins = insts[i]
            si = ins.sync_info
            mw = max_waits
            if si is not None and len(si.on_wait) > mw:
                waits = list(si.on_wait)
                extra = waits[:-mw]
                newones = []
                for j in range(0, len(extra), max_waits):  # nops take 2
                    if ins.__class__.__name__ == "InstDrain":
                        d = mybir.InstDrain(
                            name=f"{ins.name}-sw{j}", engine=ins.engine
                        )
                    else:
                        d = mybir.InstNoOp(name=f"{ins.name}-sw{j}", engine=ins.engine)
                    si2 = copy.deepcopy(si)
                    si2.on_wait = extra[j:j + max_waits]
                    si2.on_update = []
                    d.sync_info = si2
                    newones.append(d)
                si.on_wait = waits[-mw:]
                for d in reversed(newones):
                    insts.insert(i, d)
                i += len(newones)
            i += 1


def _build(plan, wtot, split=True):
    key = (tuple(plan), wtot, split)
    if key in _BUILD_CACHE:
        return _BUILD_CACHE[key]

    nc = bass.Bass()
    xT = nc.declare_dram_parameter("xT", [C, TP], BF16, isOutput=False)
    wa = nc.declare_dram_parameter("wa", [C, 3 * NHG * HD], BF16, isOutput=False)
    wp = nc.declare_dram_parameter("wp", [NHG * HD, C], BF16, isOutput=False)
    mp = nc.declare_dram_parameter("mp", [128, max(wtot, 4)], BF16, isOutput=False)
    out = nc.declare_dram_parameter("out", [T, C], BF16, isOutput=True)

    with tile.TileContext(nc) as tc:
        with ExitStack() as ctx:
            const = ctx.enter_context(tc.tile_pool(name="const", bufs=1))

            # ---- input DMAs in contraction-k order: wa[k] then xt[k] ----
            wa_sb = []
            xt_sb = []
            for k in range(KC):
                ta = const.tile([128, 3 * NHG * HD], BF16, tag=f"wa{k}", name=f"wa{k}")
                nc.sync.dma_start(out=ta[:, :], in_=wa[k * 128:(k + 1) * 128, :])
                tx = const.tile([128, TP], BF16, tag=f"xt{k}", name=f"xt{k}")
                for (q0, q1) in QCHC:
                    nc.sync.dma_start(
                        out=tx[:, q0:q1], in_=xT[k * 128:(k + 1) * 128, q0:q1]
                    )
                wa_sb.append(ta)
                xt_sb.append(tx)

            wp_sb = []
            for k in range(3):
                t_ = const.tile([128, C], BF16, tag=f"wp{k}", name=f"wp{k}")
                nc.sync.dma_start(out=t_[:, :], in_=wp[k * 128:(k + 1) * 128, :])
                wp_sb.append(t_)

            mk_sb = {}
            for ci in range(len(QCH)):
                for (kt, kw, qv0, qv1, bbox, moff) in plan[ci]:
                    if bbox is None:
                        continue
                    r0, r1, c0, c1 = bbox
                    w = c1 - c0
                    t_ = const.tile([128, w], BF16, tag=f"mk{ci}_{kt}", name=f"mk{ci}_{kt}")
                    nc.sync.dma_start(
                        out=t_[r0:r1, :], in_=mp[r0:r1, moff:moff + w]
                    )
                    mk_sb[(ci, kt)] = t_

            mones = const.tile([2, 64], BF16, tag="mones", name="mones")
            nc.vector.memset(mones[:, :], -0.5)

            qt_sb = [const.tile([128, TP], BF16, tag=f"qt{p}", name=f"qt{p}") for p in range(NPACK)]
            kt_sb = [const.tile([128, TP], BF16, tag=f"kt{p}", name=f"ktt{p}") for p in range(NPACK)]
            v6_sb = [const.tile([128, NHG * HD1], BF16, tag=f"v6{t}", name=f"v6{t}") for t in range(NKT)]
            yt_sb = [const.tile([128, TP], BF16, tag=f"yt{p}", name=f"yt{p}") for p in range(NPACK)]

            with tc.tile_pool(name="ps8", bufs=2, space="PSUM") as ps8, \
                 tc.tile_pool(name="epool", bufs=3) as epool, \
                 tc.tile_pool(name="ubp", bufs=14) as ubp, \
                 tc.tile_pool(name="npool", bufs=4) as npool, \
                 tc.tile_pool(name="osb", bufs=3) as osb:

                def big_ps():
                    # shared 2-buffer ring of 4KB (2-bank) PSUM tiles
                    return ps8.tile([128, 2, 512], F32, tag="s", name="st")

                # ---- phase emitters ------------------------------------
                def emit_qk_chunks(cis):
                    # PSUM matmul outputs must stay within one 2KB bank,
                    # so emit per QCHC sub-chunk (<=512 fp32 wide)
                    for p in range(NPACK):
                        for j, dst in ((0, qt_sb), (1, kt_sb)):
                            for ci in cis:
                                a, b = QCHC[ci]
                                k1 = b
                                if j == 1 and b > KTAIL:
                                    k1 = KTAIL  # tail K cols computed separately
                                nn = k1 - a
                                ps = big_ps().rearrange("a b c -> a (b c)")
                                col = j * NHG * HD + p * 128
                                for k in range(KC):
                                    nc.tensor.matmul(
                                        ps[:, 0:nn],
                                        wa_sb[k][:, col:col + 128],
                                        xt_sb[k][:, a:k1],
                                        start=(k == 0), stop=(k == KC - 1),
                                    )
                                nc.vector.tensor_copy(dst[p][:, a:k1], ps[:, 0:nn])

                def emit_k_tail():
                    n = TP - KTAIL  # 42
                    for p in range(NPACK):
                        ps = big_ps().rearrange("a b c -> a (b c)")
                        col = NHG * HD + p * 128
                        for k in range(KC):
                            nc.tensor.matmul(
                                ps[:, 0:n],
                                wa_sb[k][:, col:col + 128],
                                xt_sb[k][:, KTAIL:TP],
                                start=(k == 0), stop=(k == KC - 1),
                            )
                        nc.vector.tensor_copy(kt_sb[p][:, KTAIL:TP], ps[:, 0:n])

                def emit_v_tiles(ts):
                    for t in ts:
                        tw = min(128, T - t * KT)
                        twp = tw if tw % 2 == 0 else tw + 1
                        ps = big_ps().rearrange("a b c -> a (b c)")
                        for k in range(KC):
                            nc.tensor.matmul(
                                ps[0:twp, 0:NHG * HD],
                                xt_sb[k][:, t * KT:t * KT + twp],
                                wa_sb[k][:, 2 * NHG * HD:3 * NHG * HD],
                                start=(k == 0), stop=(k == KC - 1),
                            )
                        v6v = v6_sb[t].rearrange("a (h d) -> a h d", d=HD1)
                        if tw < 128:
                            nc.vector.memset(v6_sb[t][:, :], 0.0)
                        psv = ps[:, 0:NHG * HD].rearrange("a (h d) -> a h d", d=HD)
                        nc.vector.tensor_copy(v6v[0:tw, :, 0:HD], psv[0:tw, :, :])
                        nc.vector.memset(v6v[0:tw, :, HD:HD + 2], 1.0)

                pending = []   # deferred normalization closures

                def emit_attn_chunk(ci):
                    q0, q1 = QCHC[ci]
                    n = q1 - q0
                    items = plan[ci]
                    first_kt = items[0][0]
                    last_kt = items[-1][0]
                    for p in range(NPACK):
                        u2 = [ps8.tile([HD1, 448], F32, tag="u", name="ut")
                              for _ in (0, 1)]
                        sts = {}
                        ets = {}

                        def emit_s(idx):
                            kt, kw, qv0, qv1, bbox, _m = items[idx]
                            kwp = kw if kw % 2 == 0 else kw + 1
                            st = big_ps()
                            for e in (0, 1):
                                nc.tensor.matmul(
                                    st[0:kwp, e, qv0:qv1],
                                    kt_sb[p][e * 64:(e + 1) * 64,
                                             kt * KT:kt * KT + kwp],
                                    qt_sb[p][e * 64:(e + 1) * 64,
                                             q0 + qv0:q0 + qv1],
                                    start=True, stop=True,
                                )
                            sts[idx] = st

                        def emit_exp(idx):
                            kt, kw, qv0, qv1, bbox, _m = items[idx]
                            kwp = kw if kw % 2 == 0 else kw + 1
                            et = epool.tile([128, 2, 512], BF16, tag="e", name="et")
                            nc.scalar.activation(
                                et[0:kwp, :, qv0:qv1], sts[idx][0:kwp, :, qv0:qv1],
                                AF.Exp, scale=0.125,
                            )
                            ets[idx] = et

                        def emit_mask_pv(idx):
                            kt, kw, qv0, qv1, bbox, _m = items[idx]
                            kwp = kw if kw % 2 == 0 else kw + 1
                            et = ets.pop(idx)
                            sts.pop(idx)
                            if bbox is not None:
                                r0, r1, c0, c1 = bbox
                                mk = mk_sb[(ci, kt)]
                                for e in (0, 1):
                                    nc.gpsimd.tensor_mul(
                                        et[r0:r1, e, c0:c1],
                                        et[r0:r1, e, c0:c1],
                                        mk[r0:r1, 0:c1 - c0],
                                    )
                            for e in (0, 1):
                                h = 2 * p + e
                                nc.tensor.matmul(
                                    u2[e][0:HD1, qv0:qv1],
                                    v6_sb[kt][0:kwp, h * HD1:(h + 1) * HD1],
                                    et[0:kwp, e, qv0:qv1],
                                    start=(kt == first_kt), stop=(kt == last_kt),
                                    skip_group_check=(kt != first_kt),
                                )

                        # software-pipelined S -> exp -> PV
                        emit_s(0)
                        for i in range(len(items)):
                            emit_exp(i)
                            if i + 1 < len(items):
                                emit_s(i + 1)
                            emit_mask_pv(i)

                        # drain u2 quickly; defer the normalization chain
                        ubs = []
                        lns = []
                        for e in (0, 1):
                            ub = ubp.tile([HD1, 448], BF16, tag="ub", name="ub")
                            nc.vector.tensor_copy(ub[0:HD1, 0:n], u2[e][0:HD1, 0:n])
                            lnb = ubp.tile([2, 448], BF16, tag="lnb", name="lnb")
                            nc.scalar.activation(
                                lnb[0:2, 0:n], u2[e][64:66, 0:n], AF.Ln
                            )
                            ubs.append(ub)
                            lns.append(lnb)

                        def norm(p=p, ci=ci, ubs=ubs, lns=lns, n=n, q0=q0, q1=q1):
                            for e in (0, 1):
                                rb = ps8.tile([64, 448], F32, tag="rb", name="rbt")
                                nc.tensor.matmul(
                                    rb[0:64, 0:n],
                                    mones[0:2, 0:64],
                                    lns[e][0:2, 0:n],
                                    start=True, stop=True,
                                )
                                rbs = npool.tile([64, 448], BF16, tag="rbs", name="rbs")
                                nc.scalar.activation(rbs[0:64, 0:n], rb[0:64, 0:n], AF.Exp)
                                nc.gpsimd.tensor_mul(
                                    yt_sb[p][e * 64:(e + 1) * 64, q0:q1],
                                    ubs[e][0:64, 0:n],
                                    rbs[0:64, 0:n],
                                )

                        pending.append(norm)

                def emit_norms():
                    while pending:
                        pending.pop(0)()

                def emit_d_tiles(ts):
                    for t in ts:
                        tw = min(128, T - t * KT)
                        twp = tw if tw % 2 == 0 else tw + 1
                        po = big_ps()  # two 384-wide halves, one per bank
                        for hb, (n0, n1) in enumerate(((0, 384), (384, 768))):
                            for k3 in range(3):
                                nc.tensor.matmul(
                                    po[0:twp, hb, 0:384],
                                    yt_sb[k3][:, t * KT:t * KT + twp],
                                    wp_sb[k3][:, n0:n1],
                                    start=(k3 == 0), stop=(k3 == 2),
                                )
                        ot = osb.tile([128, C], BF16, tag="ot", name="ot_sb")
                        nc.vector.tensor_copy(
                            ot.rearrange("a (b c) -> a b c", c=384)[0:tw, :, :],
                            po[0:tw, :, 0:384],
                        )
                        nc.sync.dma_start(
                            out=out[t * KT:t * KT + tw, :], in_=ot[0:tw, :]
                        )

                # ---- schedule ------------------------------------------
                # D tile t needs yt chunks up to (128*(t+1)-1)//424
                d_of = {}
                for t in range(NKT):
                    d_of.setdefault(min(3, (128 * (t + 1) - 1) // 424), []).append(t)

                emit_qk_chunks([0, 1])
                emit_k_tail()
                emit_v_tiles([13, 0, 1, 2, 3])
                emit_attn_chunk(0)
                emit_qk_chunks([2, 3])
                emit_v_tiles(range(4, 13))
                emit_attn_chunk(1)
                emit_norms()          # norms for chunks 0,1
                emit_attn_chunk(2)
                emit_d_tiles(d_of[0])
                emit_d_tiles(d_of[1])
                emit_attn_chunk(3)
                emit_norms()          # norms for chunks 2,3
                emit_d_tiles(d_of[2])
                emit_d_tiles(d_of[3])

    if split:
        _split_excess_waits(nc)
    _BUILD_CACHE[key] = nc
    return nc


def _prep_inputs(x, W_attn, W_proj, mpack):
    """Per-core input maps. core c -> batch c//2, head-group c%2."""
    x = np.asarray(x, np.float32)
    W_attn = np.asarray(W_attn, np.float32)
    W_proj = np.asarray(W_proj, np.float32)
    mpack_bf = mpack.astype(NPBF16)
    in_maps = []
    xT_by_batch = []
    for b in range(B):
        xt = np.zeros((C, TP), NPBF16)
        xt[:, :T] = x[b][PERM, :].T.astype(NPBF16)
        xT_by_batch.append(xt)
    for c in range(NCORES):
        b, g = c // 2, c % 2
        cs = slice(g * NHG * HD, (g + 1) * NHG * HD)
        wa_s = np.ascontiguousarray(
            np.concatenate(
                [W_attn[:, cs], W_attn[:, C:][:, cs], W_attn[:, 2 * C:][:, cs]],
                axis=1,
            ).astype(NPBF16)
        )
        wp_s = np.ascontiguousarray(W_proj[cs, :].astype(NPBF16))
        in_maps.append(
            {"xT": xT_by_batch[b], "wa": wa_s, "wp": wp_s, "mp": mpack_bf}
        )
    return in_maps


def _run(inputs, trace=False, trace_cores=None):
    x = np.asarray(inputs["x"], np.float32)
    mask = np.asarray(inputs["mask"], bool)
    mask_perm = mask[np.ix_(PERM, PERM)]
    plan, mpack = _analyze(mask_perm)
    nc = _build(plan, mpack.shape[1])
    in_maps = _prep_inputs(x, inputs["W_attn"], inputs["W_proj"], mpack)
    res = run_bass_kernel_spmd(
        nc, in_maps, list(range(NCORES)), trace=trace, trace_cores=trace_cores
    )
    outs = [np.asarray(r["out"]).astype(np.float32) for r in res.results]
    full = np.empty((B, T, C), np.float32)
    for b in range(B):
        comb = outs[2 * b] + outs[2 * b + 1]
        full[b][PERM, :] = comb
    return full, res


def kernel(**inputs) -> np.ndarray:
    out, _ = _run(inputs)
    return out


# revision 16
# speedup vs baseline: 1.4408x; 1.0027x over previous
"""Block-sparse causal self-attention on 8 TRN2 NeuronCores (SPMD Bass/Tile kernel).

Sharding: core c -> (batch b = c//2, head-group g = c%2 of 6 heads).
Each core computes qkv projection (its 6 heads), masked attention, and a
partial output projection (its 384 rows of W_proj).  Host sums the two
partials per batch and concatenates batches.

Token reorder (host-side permutation, inverted on output):
  [U_0 .. U_7 | A]  with U_i = [tactile_i (16), image_i (196)], A = 9 actions.
This makes the block-sparse mask nearly block-lower-triangular with
frame-aligned boundaries; the few partial tiles get an elementwise
multiply restricted to the bounding box of their masked region.

Attention is computed in transposed layout S^T[k, q]; softmax
normalization comes from ones-columns appended to V (rowsum lands in the
PV matmul output), a fast DVE reciprocal, and a tiny ones-matmul that
broadcasts 1/rowsum across 64 partitions.

v3 structure (all matmul operands bf16, fp32 PSUM):
 - program order interleaves projection / attention / output phases per
   query chunk so the ACT (exp) and DVE engines start within ~10us and
   no phase serializes the whole kernel;
 - within a chunk the S->exp->PV chain is software-pipelined: S(kt+1)
   issues between exp(kt) and PV(kt) so the tensor engine never waits
   on the activation engine;
 - softmax normalization is deferred by one chunk so its cross-engine
   chain never blocks the in-order tensor queue;
 - action-token K columns and V tile 13 are computed up front (the
   permutation puts action keys last, and every chunk attends to them);
 - one shared 2-buffer PSUM ring (4 banks) serves S tiles, projection
   groups and output-projection groups; u2 accumulators and the
   broadcast tile use the remaining 4 banks.
"""

import os
import sys
from contextlib import ExitStack

import numpy as np

for _p in ("/opt/trn_rl_repo", "/root/.axon_site/_ro/trn_rl_repo"):
    if os.path.isdir(_p) and _p not in sys.path:
        sys.path.insert(0, _p)

import concourse.bass as bass
import concourse.tile as tile
from concourse import mybir
from concourse.bass_utils import run_bass_kernel_spmd

F32 = mybir.dt.float32
BF16 = mybir.dt.bfloat16
FP16 = mybir.dt.float16
NPBF16 = mybir.dt.np(BF16)
AF = mybir.ActivationFunctionType

L, PP, PT = 8, 196, 16
T, C, NH, B, HD = 1705, 768, 12, 4, 64
NCORES = 8
NHG = NH // 2          # heads per core = 6
NPACK = NHG // 2       # head pairs per core = 3
KC = C // 128          # 6 contraction tiles over C
KT = 128               # key tile size
NKT = (T + KT - 1) // KT   # 14
TP = 1706              # T padded to even
QCH = [(0, 424), (424, 848), (848, 1272), (1272, T)]
QCHC = [(0, 424), (424, 848), (848, 1272), (1272, TP)]  # compute chunks (even n)
HD1 = HD + 2           # V width: 64 V cols | ones col | ones col
KTAIL = 1664           # action-key tail start (tile 13), computed up front
QKB = [(0, 854), (854, 1706)]  # projection chunks


def _perm():
    idx = []
    for i in range(L):
        idx += list(range(9 + PT * i, 9 + PT * (i + 1)))
        idx += list(range(9 + L * PT + PP * i, 9 + L * PT + PP * (i + 1)))
    idx += list(range(0, 9))
    return np.asarray(idx, dtype=np.int64)


PERM = _perm()


def _analyze(mask_perm):
    """Compile-time plan from the (permuted) boolean mask.

    Returns (plan, mpack):
      plan: per query-chunk, tuple of (kt, kw, qv0, qv1, bbox-or-None, moff):
            qv0/qv1 = chunk-relative visible query span (even-aligned);
            bbox = (r0, r1, c0, c1) of the masked (zero) region inside the
            S^T tile [kw keys x chunk queries], clipped to the span;
            moff = column offset into mpack.
      mpack: [128, Wtot] float32 packed mask bounding boxes (S^T layout).
    """
    plan = []
    cols = []
    widths = 0
    for (q0, q1) in QCH:
        sub = mask_perm[q0:q1, :]
        nq = sub.shape[0]
        items = []
        for kt in range(NKT):
            k0, k1 = kt * KT, min((kt + 1) * KT, T)
            m = sub[:, k0:k1]
            if not m.any():
                continue
            kw = k1 - k0
            qv = np.nonzero(m.any(axis=1))[0]
            qv0 = int(qv[0]) & ~1
            qv1 = min(nq + (nq & 1), (int(qv[-1]) + 2) & ~1)
            if m.all():
                items.append((kt, kw, qv0, qv1, None, 0))
            else:
                mt = m.T  # [kw, nq]  S^T layout
                z = ~mt
                rr = np.nonzero(z.any(axis=1))[0]
                cc = np.nonzero(z.any(axis=0))[0]
                r0, r1 = int(rr[0]), int(rr[-1]) + 1
                # engine partition windows: start 0 (any count) or 64 (<=64)
                r0 = 0 if r0 < 64 else 64
                c0 = max(int(cc[0]), qv0)
                c1 = min(int(cc[-1]) + 1, qv1)
                if c0 >= c1:
                    items.append((kt, kw, qv0, qv1, None, 0))
                    continue
                tilefrag = np.ones((128, c1 - c0), np.float32)
                tilefrag[r0:r1, :] = mt[r0:r1, c0:c1].astype(np.float32)
                items.append((kt, kw, qv0, qv1, (r0, r1, c0, c1), widths))
                cols.append(tilefrag)
                widths += c1 - c0
        # PSUM accumulation relies on the first tile covering the full chunk
        assert items[0][2] == 0 and items[0][3] >= nq
        plan.append(tuple(items))
    if widths == 0:
        mpack = np.zeros((128, 4), np.float32)
    else:
        mpack = np.concatenate(cols, axis=1)
    return tuple(plan), np.ascontiguousarray(mpack)


_BUILD_CACHE = {}


def _split_excess_waits(nc, max_waits=1):
    """walrus (this build) rejects instructions with >2 sem-wait commands.

    Tile's kernel-tail drain waits on every live semaphore in one Drain;
    split the excess onto preceding same-engine instructions (extra Drains
    for InstDrain, NoOps otherwise).
    """
    import copy

    for bb in nc.main_func.blocks:
        insts = bb.instructions
        i = 0
        while i < len(insts):
            ins = insts[i]
            si = ins.sync_info
            mw = max_waits
            if si is not None and len(si.on_wait) > mw:
                waits = list(si.on_wait)
                extra = waits[:-mw]
                newones = []
                for j in range(0, len(extra), max_waits):  # nops take 2
                    if ins.__class__.__name__ == "InstDrain":
                        d = mybir.InstDrain(
                            name=f"{ins.name}-sw{j}", engine=ins.engine
                        )
                    else:
                        d = mybir.InstNoOp(name=f"{ins.name}-sw{j}", engine=ins.engine)
                    si2 = copy.deepcopy(si)
                    si2.on_wait = extra[j:j + max_waits]
                    si2.on_update = []
                    d.sync_info = si2
                    newones.append(d)
                si.on_wait = waits[-mw:]
                for d in reversed(newones):
                    insts.insert(i, d)
                i += len(newones)
            i += 1


def _build(plan, wtot, split=True):
    key = (tuple(plan), wtot, split)
    if key in _BUILD_CACHE:
        return _BUILD_CACHE[key]

    nc = bass.Bass()
    xT = nc.declare_dram_parameter("xT", [C, TP], BF16, isOutput=False)
    wa = nc.declare_dram_parameter("wa", [C, 3 * NHG * HD], BF16, isOutput=False)
    wp = nc.declare_dram_parameter("wp", [NHG * HD, C], BF16, isOutput=False)
    mp = nc.declare_dram_parameter("mp", [128, max(wtot, 4)], BF16, isOutput=False)
    out = nc.declare_dram_parameter("out", [T, C], BF16, isOutput=True)

    with tile.TileContext(nc) as tc:
        with ExitStack() as ctx:
            const = ctx.enter_context(tc.tile_pool(name="const", bufs=1))

            # ---- input DMAs in contraction-k order: wa[k] then xt[k] ----
            wa_sb = []
            xt_sb = []
            for k in range(KC):
                ta = const.tile([128, 3 * NHG * HD], BF16, tag=f"wa{k}", name=f"wa{k}")
                nc.sync.dma_start(out=ta[:, :], in_=wa[k * 128:(k + 1) * 128, :])
                tx = const.tile([128, TP], BF16, tag=f"xt{k}", name=f"xt{k}")
                for (q0, q1) in QCHC:
                    nc.sync.dma_start(
                        out=tx[:, q0:q1], in_=xT[k * 128:(k + 1) * 128, q0:q1]
                    )
                wa_sb.append(ta)
                xt_sb.append(tx)

            wp_sb = []
            for k in range(3):
                t_ = const.tile([128, C], BF16, tag=f"wp{k}", name=f"wp{k}")
                nc.sync.dma_start(out=t_[:, :], in_=wp[k * 128:(k + 1) * 128, :])
                wp_sb.append(t_)

            mk_sb = {}
            for ci in range(len(QCH)):
                for (kt, kw, qv0, qv1, bbox, moff) in plan[ci]:
                    if bbox is None:
                        continue
                    r0, r1, c0, c1 = bbox
                    w = c1 - c0
                    t_ = const.tile([128, w], BF16, tag=f"mk{ci}_{kt}", name=f"mk{ci}_{kt}")
                    nc.sync.dma_start(
                        out=t_[r0:r1, :], in_=mp[r0:r1, moff:moff + w]
                    )
                    mk_sb[(ci, kt)] = t_

            mones = const.tile([2, 64], FP16, tag="mones", name="mones")
            nc.vector.memset(mones[:, :], -0.5)

            qt_sb = [const.tile([128, TP], BF16, tag=f"qt{p}", name=f"qt{p}") for p in range(NPACK)]
            kt_sb = [const.tile([128, TP], BF16, tag=f"kt{p}", name=f"ktt{p}") for p in range(NPACK)]
            v6_sb = [const.tile([128, NHG * HD1], BF16, tag=f"v6{t}", name=f"v6{t}") for t in range(NKT)]
            yt_sb = [const.tile([128, TP], BF16, tag=f"yt{p}", name=f"yt{p}") for p in range(NPACK)]

            with tc.tile_pool(name="ps8", bufs=2, space="PSUM") as ps8, \
                 tc.tile_pool(name="epool", bufs=3) as epool, \
                 tc.tile_pool(name="ubp", bufs=14) as ubp, \
                 tc.tile_pool(name="npool", bufs=4) as npool, \
                 tc.tile_pool(name="osb", bufs=3) as osb:

                def big_ps():
                    # shared 2-buffer ring of 4KB (2-bank) PSUM tiles
                    return ps8.tile([128, 2, 512], F32, tag="s", name="st")

                # ---- phase emitters ------------------------------------
                def emit_qk_chunks(cis):
                    # PSUM matmul outputs must stay within one 2KB bank,
                    # so emit per QCHC sub-chunk (<=512 fp32 wide)
                    for p in range(NPACK):
                        for j, dst in ((0, qt_sb), (1, kt_sb)):
                            for ci in cis:
                                a, b = QCHC[ci]
                                k1 = b
                                if j == 1 and b > KTAIL:
                                    k1 = KTAIL  # tail K cols computed separately
                                nn = k1 - a
                                ps = big_ps().rearrange("a b c -> a (b c)")
                                col = j * NHG * HD + p * 128
                                for k in range(KC):
                                    nc.tensor.matmul(
                                        ps[:, 0:nn],
                                        wa_sb[k][:, col:col + 128],
                                        xt_sb[k][:, a:k1],
                                        start=(k == 0), stop=(k == KC - 1),
                                    )
                                nc.vector.tensor_copy(dst[p][:, a:k1], ps[:, 0:nn])

                def emit_k_tail():
                    n = TP - KTAIL  # 42
                    for p in range(NPACK):
                        ps = big_ps().rearrange("a b c -> a (b c)")
                        col = NHG * HD + p * 128
                        for k in range(KC):
                            nc.tensor.matmul(
                                ps[:, 0:n],
                                wa_sb[k][:, col:col + 128],
                                xt_sb[k][:, KTAIL:TP],
                                start=(k == 0), stop=(k == KC - 1),
                            )
                        nc.vector.tensor_copy(kt_sb[p][:, KTAIL:TP], ps[:, 0:n])

                def emit_v_tiles(ts):
                    for t in ts:
                        tw = min(128, T - t * KT)
                        twp = tw if tw % 2 == 0 else tw + 1
                        ps = big_ps().rearrange("a b c -> a (b c)")
                        for k in range(KC):
                            nc.tensor.matmul(
                                ps[0:twp, 0:NHG * HD],
                                xt_sb[k][:, t * KT:t * KT + twp],
                                wa_sb[k][:, 2 * NHG * HD:3 * NHG * HD],
                                start=(k == 0), stop=(k == KC - 1),
                            )
                        v6v = v6_sb[t].rearrange("a (h d) -> a h d", d=HD1)
                        if tw < 128:
                            nc.vector.memset(v6_sb[t][:, :], 0.0)
                        psv = ps[:, 0:NHG * HD].rearrange("a (h d) -> a h d", d=HD)
                        nc.vector.tensor_copy(v6v[0:tw, :, 0:HD], psv[0:tw, :, :])
                        nc.vector.memset(v6v[0:tw, :, HD:HD + 2], 1.0)

                pending = []   # deferred normalization closures

                def emit_attn_chunk(ci):
                    q0, q1 = QCHC[ci]
                    n = q1 - q0
                    items = plan[ci]
                    first_kt = items[0][0]
                    last_kt = items[-1][0]
                    for p in range(NPACK):
                        u2 = [ps8.tile([HD1, 448], F32, tag="u", name="ut")
                              for _ in (0, 1)]
                        sts = {}
                        ets = {}

                        def emit_s(idx):
                            kt, kw, qv0, qv1, bbox, _m = items[idx]
                            kwp = kw if kw % 2 == 0 else kw + 1
                            st = big_ps()
                            for e in (0, 1):
                                nc.tensor.matmul(
                                    st[0:kwp, e, qv0:qv1],
                                    kt_sb[p][e * 64:(e + 1) * 64,
                                             kt * KT:kt * KT + kwp],
                                    qt_sb[p][e * 64:(e + 1) * 64,
                                             q0 + qv0:q0 + qv1],
                                    start=True, stop=True,
                                )
                            sts[idx] = st

                        def emit_exp(idx):
                            kt, kw, qv0, qv1, bbox, _m = items[idx]
                            kwp = kw if kw % 2 == 0 else kw + 1
                            et = epool.tile([128, 2, 512], BF16, tag="e", name="et")
                            nc.scalar.activation(
                                et[0:kwp, :, qv0:qv1], sts[idx][0:kwp, :, qv0:qv1],
                                AF.Exp, scale=0.125,
                            )
                            ets[idx] = et

                        def emit_mask_pv(idx):
                            kt, kw, qv0, qv1, bbox, _m = items[idx]
                            kwp = kw if kw % 2 == 0 else kw + 1
                            et = ets.pop(idx)
                            sts.pop(idx)
                            if bbox is not None:
                                r0, r1, c0, c1 = bbox
                                mk = mk_sb[(ci, kt)]
                                for e in (0, 1):
                                    nc.gpsimd.tensor_mul(
                                        et[r0:r1, e, c0:c1],
                                        et[r0:r1, e, c0:c1],
                                        mk[r0:r1, 0:c1 - c0],
                                    )
                            for e in (0, 1):
                                h = 2 * p + e
                                nc.tensor.matmul(
                                    u2[e][0:HD1, qv0:qv1],
                                    v6_sb[kt][0:kwp, h * HD1:(h + 1) * HD1],
                                    et[0:kwp, e, qv0:qv1],
                                    start=(kt == first_kt), stop=(kt == last_kt),
                                    skip_group_check=(kt != first_kt),
                                )

                        # software-pipelined S -> exp -> PV
                        emit_s(0)
                        for i in range(len(items)):
                            emit_exp(i)
                            if i + 1 < len(items):
                                emit_s(i + 1)
                            emit_mask_pv(i)

                        # drain u2 quickly; defer the normalization chain
                        ubs = []
                        lns = []
                        for e in (0, 1):
                            ub = ubp.tile([HD1, 448], BF16, tag="ub", name="ub")
                            nc.vector.tensor_copy(ub[0:HD1, 0:n], u2[e][0:HD1, 0:n])
                            lnb = ubp.tile([2, 448], FP16, tag="lnb", name="lnb")
                            nc.scalar.activation(
                                lnb[0:2, 0:n], u2[e][64:66, 0:n], AF.Ln
                            )
                            ubs.append(ub)
                            lns.append(lnb)

                        def norm(p=p, ci=ci, ubs=ubs, lns=lns, n=n, q0=q0, q1=q1):
                            for e in (0, 1):
                                rb = ps8.tile([64, 448], F32, tag="rb", name="rbt")
                                nc.tensor.matmul(
                                    rb[0:64, 0:n],
                                    mones[0:2, 0:64],
                                    lns[e][0:2, 0:n],
                                    start=True, stop=True,
                                )
                                rbs = npool.tile([64, 448], BF16, tag="rbs", name="rbs")
                                nc.scalar.activation(rbs[0:64, 0:n], rb[0:64, 0:n], AF.Exp)
                                nc.gpsimd.tensor_mul(
                                    yt_sb[p][e * 64:(e + 1) * 64, q0:q1],
                                    ubs[e][0:64, 0:n],
                                    rbs[0:64, 0:n],
                                )

                        pending.append(norm)

                def emit_norms():
                    while pending:
                        pending.pop(0)()

                def emit_d_tiles(ts):
                    for t in ts:
                        tw = min(128, T - t * KT)
                        twp = tw if tw % 2 == 0 else tw + 1
                        po = big_ps()  # two 384-wide halves, one per bank
                        for hb, (n0, n1) in enumerate(((0, 384), (384, 768))):
                            for k3 in range(3):
                                nc.tensor.matmul(
                                    po[0:twp, hb, 0:384],
                                    yt_sb[k3][:, t * KT:t * KT + twp],
                                    wp_sb[k3][:, n0:n1],
                                    start=(k3 == 0), stop=(k3 == 2),
                                )
                        ot = osb.tile([128, C], BF16, tag="ot", name="ot_sb")
                        nc.vector.tensor_copy(
                            ot.rearrange("a (b c) -> a b c", c=384)[0:tw, :, :],
                            po[0:tw, :, 0:384],
                        )
                        nc.sync.dma_start(
                            out=out[t * KT:t * KT + tw, :], in_=ot[0:tw, :]
                        )

                # ---- schedule ------------------------------------------
                # D tile t needs yt chunks up to (128*(t+1)-1)//424
                d_of = {}
                for t in range(NKT):
                    d_of.setdefault(min(3, (128 * (t + 1) - 1) // 424), []).append(t)

                emit_qk_chunks([0, 1])
                emit_k_tail()
                emit_v_tiles([13, 0, 1, 2, 3])
                emit_attn_chunk(0)
                emit_qk_chunks([2, 3])
                emit_v_tiles(range(4, 13))
                emit_attn_chunk(1)
                emit_norms()          # norms for chunks 0,1
                emit_attn_chunk(2)
                emit_d_tiles(d_of[0])
                emit_d_tiles(d_of[1])
                emit_attn_chunk(3)
                emit_norms()          # norms for chunks 2,3
                emit_d_tiles(d_of[2])
                emit_d_tiles(d_of[3])

    if split:
        _split_excess_waits(nc)
    _BUILD_CACHE[key] = nc
    return nc


def _prep_inputs(x, W_attn, W_proj, mpack):
    """Per-core input maps. core c -> batch c//2, head-group c%2."""
    x = np.asarray(x, np.float32)
    W_attn = np.asarray(W_attn, np.float32)
    W_proj = np.asarray(W_proj, np.float32)
    mpack_bf = mpack.astype(NPBF16)
    in_maps = []
    xT_by_batch = []
    for b in range(B):
        xt = np.zeros((C, TP), NPBF16)
        xt[:, :T] = x[b][PERM, :].T.astype(NPBF16)
        xT_by_batch.append(xt)
    for c in range(NCORES):
        b, g = c // 2, c % 2
        cs = slice(g * NHG * HD, (g + 1) * NHG * HD)
        wa_s = np.ascontiguousarray(
            np.concatenate(
                [W_attn[:, cs], W_attn[:, C:][:, cs], W_attn[:, 2 * C:][:, cs]],
                axis=1,
            ).astype(NPBF16)
        )
        wp_s = np.ascontiguousarray(W_proj[cs, :].astype(NPBF16))
        in_maps.append(
            {"xT": xT_by_batch[b], "wa": wa_s, "wp": wp_s, "mp": mpack_bf}
        )
    return in_maps


def _run(inputs, trace=False, trace_cores=None):
    x = np.asarray(inputs["x"], np.float32)
    mask = np.asarray(inputs["mask"], bool)
    mask_perm = mask[np.ix_(PERM, PERM)]
    plan, mpack = _analyze(mask_perm)
    nc = _build(plan, mpack.shape[1])
    in_maps = _prep_inputs(x, inputs["W_attn"], inputs["W_proj"], mpack)
    res = run_bass_kernel_spmd(
        nc, in_maps, list(range(NCORES)), trace=trace, trace_cores=trace_cores
    )
    outs = [np.asarray(r["out"]).astype(np.float32) for r in res.results]
    full = np.empty((B, T, C), np.float32)
    for b in range(B):
        comb = outs[2 * b] + outs[2 * b + 1]
        full[b][PERM, :] = comb
    return full, res


def kernel(**inputs) -> np.ndarray:
    out, _ = _run(inputs)
    return out


# revision 22
# speedup vs baseline: 1.5836x; 1.0991x over previous
"""Block-sparse causal self-attention on 8 TRN2 NeuronCores (SPMD Bass/Tile kernel).

Sharding: core c -> (batch b = c//2, head-group g = c%2 of 6 heads).
Each core computes qkv projection (its 6 heads), masked attention, and a
partial output projection (its 384 rows of W_proj).  Host sums the two
partials per batch and concatenates batches.

Token reorder (host-side permutation, inverted on output):
  [U_0 .. U_7 | A]  with U_i = [tactile_i (16), image_i (196)], A = 9 actions.
This makes the block-sparse mask nearly block-lower-triangular with
frame-aligned boundaries; the few partial tiles get an elementwise
multiply restricted to the bounding box of their masked region.

Attention is computed in transposed layout S^T[k, q]; softmax
normalization comes from ones-columns appended to V (rowsum lands in the
PV matmul output), a fast DVE reciprocal, and a tiny ones-matmul that
broadcasts 1/rowsum across 64 partitions.

v3 structure (all matmul operands bf16, fp32 PSUM):
 - program order interleaves projection / attention / output phases per
   query chunk so the ACT (exp) and DVE engines start within ~10us and
   no phase serializes the whole kernel;
 - within a chunk the S->exp->PV chain is software-pipelined: S(kt+1)
   issues between exp(kt) and PV(kt) so the tensor engine never waits
   on the activation engine;
 - softmax normalization is deferred by one chunk so its cross-engine
   chain never blocks the in-order tensor queue;
 - action-token K columns and V tile 13 are computed up front (the
   permutation puts action keys last, and every chunk attends to them);
 - one shared 2-buffer PSUM ring (4 banks) serves S tiles, projection
   groups and output-projection groups; u2 accumulators and the
   broadcast tile use the remaining 4 banks.
"""

import os
import sys
from contextlib import ExitStack

import numpy as np

for _p in ("/opt/trn_rl_repo", "/root/.axon_site/_ro/trn_rl_repo"):
    if os.path.isdir(_p) and _p not in sys.path:
        sys.path.insert(0, _p)

import concourse.bass as bass
import concourse.tile as tile
from concourse import mybir
from concourse.bass_utils import run_bass_kernel_spmd

F32 = mybir.dt.float32
BF16 = mybir.dt.bfloat16
FP16 = mybir.dt.float16
NPBF16 = mybir.dt.np(BF16)
AF = mybir.ActivationFunctionType

L, PP, PT = 8, 196, 16
T, C, NH, B, HD = 1705, 768, 12, 4, 64
NCORES = 8
NHG = NH // 2          # heads per core = 6
NPACK = NHG // 2       # head pairs per core = 3
KC = C // 128          # 6 contraction tiles over C
KT = 128               # key tile size
NKT = (T + KT - 1) // KT   # 14
TP = 1706              # T padded to even
QCH = [(0, 424), (424, 848), (848, 1272), (1272, T)]
QCHC = [(0, 424), (424, 848), (848, 1272), (1272, TP)]  # compute chunks (even n)
HD1 = HD + 2           # V width: 64 V cols | ones col | ones col
KTAIL = 1664           # action-key tail start (tile 13), computed up front
QKB = [(0, 854), (854, 1706)]  # projection chunks


def _perm():
    idx = []
    for i in range(L):
        idx += list(range(9 + PT * i, 9 + PT * (i + 1)))
        idx += list(range(9 + L * PT + PP * i, 9 + L * PT + PP * (i + 1)))
    idx += list(range(0, 9))
    return np.asarray(idx, dtype=np.int64)


PERM = _perm()


def _analyze(mask_perm):
    """Compile-time plan from the (permuted) boolean mask.

    Returns (plan, mpack):
      plan: per query-chunk, tuple of (kt, kw, qv0, qv1, bbox-or-None, moff):
            qv0/qv1 = chunk-relative visible query span (even-aligned);
            bbox = (r0, r1, c0, c1) of the masked (zero) region inside the
            S^T tile [kw keys x chunk queries], clipped to the span;
            moff = column offset into mpack.
      mpack: [128, Wtot] float32 packed mask bounding boxes (S^T layout).
    """
    plan = []
    cols = []
    widths = 0
    for (q0, q1) in QCH:
        sub = mask_perm[q0:q1, :]
        nq = sub.shape[0]
        items = []
        for kt in range(NKT):
            k0, k1 = kt * KT, min((kt + 1) * KT, T)
            m = sub[:, k0:k1]
            if not m.any():
                continue
            kw = k1 - k0
            qv = np.nonzero(m.any(axis=1))[0]
            qv0 = int(qv[0]) & ~1
            qv1 = min(nq + (nq & 1), (int(qv[-1]) + 2) & ~1)
            if m.all():
                items.append((kt, kw, qv0, qv1, None, 0))
            else:
                mt = m.T  # [kw, nq]  S^T layout
                z = ~mt
                rr = np.nonzero(z.any(axis=1))[0]
                cc = np.nonzero(z.any(axis=0))[0]
                r0, r1 = int(rr[0]), int(rr[-1]) + 1
                # engine partition windows: start 0 (any count) or 64 (<=64)
                r0 = 0 if r0 < 64 else 64
                c0 = max(int(cc[0]), qv0)
                c1 = min(int(cc[-1]) + 1, qv1)
                if c0 >= c1:
                    items.append((kt, kw, qv0, qv1, None, 0))
                    continue
                tilefrag = np.ones((128, c1 - c0), np.float32)
                tilefrag[r0:r1, :] = mt[r0:r1, c0:c1].astype(np.float32)
                items.append((kt, kw, qv0, qv1, (r0, r1, c0, c1), widths))
                cols.append(tilefrag)
                widths += c1 - c0
        # PSUM accumulation relies on the first tile covering the full chunk
        assert items[0][2] == 0 and items[0][3] >= nq
        plan.append(tuple(items))
    if widths == 0:
        mpack = np.zeros((128, 4), np.float32)
    else:
        mpack = np.concatenate(cols, axis=1)
    return tuple(plan), np.ascontiguousarray(mpack)


_BUILD_CACHE = {}


def _split_excess_waits(nc, max_waits=1):
    """walrus (this build) rejects instructions with >2 sem-wait commands.

    Tile's kernel-tail drain waits on every live semaphore in one Drain;
    split the excess onto preceding same-engine instructions (extra Drains
    for InstDrain, NoOps otherwise).
    """
    import copy

    for bb in nc.main_func.blocks:
        insts = bb.instructions
        i = 0
        while i < len(insts):
            ins = insts[i]
            si = ins.sync_info
            mw = max_waits
            if si is not None and len(si.on_wait) > mw:
                waits = list(si.on_wait)
                extra = waits[:-mw]
                newones = []
                for j in range(0, len(extra), max_waits):  # nops take 2
                    if ins.__class__.__name__ == "InstDrain":
                        d = mybir.InstDrain(
                            name=f"{ins.name}-sw{j}", engine=ins.engine
                        )
                    else:
                        d = mybir.InstNoOp(name=f"{ins.name}-sw{j}", engine=ins.engine)
                    si2 = copy.deepcopy(si)
                    si2.on_wait = extra[j:j + max_waits]
                    si2.on_update = []
                    d.sync_info = si2
                    newones.append(d)
                si.on_wait = waits[-mw:]
                for d in reversed(newones):
                    insts.insert(i, d)
                i += len(newones)
            i += 1


def _build(plan, wtot, split=True):
    key = (tuple(plan), wtot, split)
    if key in _BUILD_CACHE:
        return _BUILD_CACHE[key]

    nc = bass.Bass()
    xT = nc.declare_dram_parameter("xT", [C, TP], BF16, isOutput=False)
    wa = nc.declare_dram_parameter("wa", [C, 3 * NHG * HD], BF16, isOutput=False)
    wp = nc.declare_dram_parameter("wp", [NHG * HD, C], BF16, isOutput=False)
    mp = nc.declare_dram_parameter("mp", [128, max(wtot, 4)], BF16, isOutput=False)
    out = nc.declare_dram_parameter("out", [T, C], BF16, isOutput=True)

    with tile.TileContext(nc) as tc:
        with ExitStack() as ctx:
            const = ctx.enter_context(tc.tile_pool(name="const", bufs=1))

            # ---- input DMAs, priority-ordered: q/k weight cols + x chunk 0
            # first (first projection groups depend only on those), then v
            # cols + later x chunks ----
            wa_sb = [
                const.tile([128, 3 * NHG * HD], BF16, tag=f"wa{k}", name=f"wa{k}")
                for k in range(KC)
            ]
            xt_sb = [
                const.tile([128, TP], BF16, tag=f"xt{k}", name=f"xt{k}")
                for k in range(KC)
            ]
            W1 = NHG * HD
            for k in range(KC):
                nc.sync.dma_start(
                    out=wa_sb[k][:, 0:2 * W1],
                    in_=wa[k * 128:(k + 1) * 128, 0:2 * W1],
                )
                q0, q1 = QCHC[0]
                nc.sync.dma_start(
                    out=xt_sb[k][:, q0:q1], in_=xT[k * 128:(k + 1) * 128, q0:q1]
                )
            for k in range(KC):
                nc.sync.dma_start(
                    out=wa_sb[k][:, 2 * W1:3 * W1],
                    in_=wa[k * 128:(k + 1) * 128, 2 * W1:3 * W1],
                )
                q0, q1 = QCHC[1]
                nc.sync.dma_start(
                    out=xt_sb[k][:, q0:q1], in_=xT[k * 128:(k + 1) * 128, q0:q1]
                )
            for (q0, q1) in QCHC[2:]:
                for k in range(KC):
                    nc.sync.dma_start(
                        out=xt_sb[k][:, q0:q1], in_=xT[k * 128:(k + 1) * 128, q0:q1]
                    )

            wp_sb = []
            for k in range(3):
                t_ = const.tile([128, C], BF16, tag=f"wp{k}", name=f"wp{k}")
                nc.sync.dma_start(out=t_[:, :], in_=wp[k * 128:(k + 1) * 128, :])
                wp_sb.append(t_)

            mk_sb = {}
            for ci in range(len(QCH)):
                for (kt, kw, qv0, qv1, bbox, moff) in plan[ci]:
                    if bbox is None:
                        continue
                    r0, r1, c0, c1 = bbox
                    w = c1 - c0
                    t_ = const.tile([128, w], BF16, tag=f"mk{ci}_{kt}", name=f"mk{ci}_{kt}")
                    nc.sync.dma_start(
                        out=t_[r0:r1, :], in_=mp[r0:r1, moff:moff + w]
                    )
                    mk_sb[(ci, kt)] = t_

            mones = const.tile([2, 64], FP16, tag="mones", name="mones")
            nc.vector.memset(mones[:, :], -0.5)

            qt_sb = [const.tile([128, TP], BF16, tag=f"qt{p}", name=f"qt{p}") for p in range(NPACK)]
            kt_sb = [const.tile([128, TP], BF16, tag=f"kt{p}", name=f"ktt{p}") for p in range(NPACK)]
            v6_sb = [const.tile([128, NHG * HD1], BF16, tag=f"v6{t}", name=f"v6{t}") for t in range(NKT)]
            yt_sb = [const.tile([128, TP], BF16, tag=f"yt{p}", name=f"yt{p}") for p in range(NPACK)]

            with tc.tile_pool(name="ps8", bufs=2, space="PSUM") as ps8, \
                 tc.tile_pool(name="epool", bufs=3) as epool, \
                 tc.tile_pool(name="ubp", bufs=14) as ubp, \
                 tc.tile_pool(name="npool", bufs=4) as npool, \
                 tc.tile_pool(name="osb", bufs=3) as osb:

                def big_ps():
                    # shared 3-buffer ring of 4KB (2-bank) PSUM tiles
                    return ps8.tile([128, 2, 512], F32, tag="s", name="st", bufs=3)

                # ---- phase emitters ------------------------------------
                def emit_qk_chunks(cis):
                    # PSUM matmul outputs must stay within one 2KB bank,
                    # so emit per QCHC sub-chunk (<=512 fp32 wide)
                    for p in range(NPACK):
                        for j, dst in ((0, qt_sb), (1, kt_sb)):
                            for ci in cis:
                                a, b = QCHC[ci]
                                k1 = b
                                if j == 1 and b > KTAIL:
                                    k1 = KTAIL  # tail K cols computed separately
                                nn = k1 - a
                                ps = big_ps().rearrange("a b c -> a (b c)")
                                col = j * NHG * HD + p * 128
                                for k in range(KC):
                                    nc.tensor.matmul(
                                        ps[:, 0:nn],
                                        wa_sb[k][:, col:col + 128],
                                        xt_sb[k][:, a:k1],
                                        start=(k == 0), stop=(k == KC - 1),
                                    )
                                if j == 0:
                                    # ACT is idle during projection windows;
                                    # offload Q copies there
                                    nc.scalar.activation(
                                        dst[p][:, a:k1], ps[:, 0:nn], AF.Copy
                                    )
                                else:
                                    nc.vector.tensor_copy(dst[p][:, a:k1], ps[:, 0:nn])

                def emit_k_tail():
                    n = TP - KTAIL  # 42
                    for p in range(NPACK):
                        ps = big_ps().rearrange("a b c -> a (b c)")
                        col = NHG * HD + p * 128
                        for k in range(KC):
                            nc.tensor.matmul(
                                ps[:, 0:n],
                                wa_sb[k][:, col:col + 128],
                                xt_sb[k][:, KTAIL:TP],
                                start=(k == 0), stop=(k == KC - 1),
                            )
                        nc.vector.tensor_copy(kt_sb[p][:, KTAIL:TP], ps[:, 0:n])

                def emit_v_tiles(ts):
                    for t in ts:
                        tw = min(128, T - t * KT)
                        twp = tw if tw % 2 == 0 else tw + 1
                        ps = big_ps().rearrange("a b c -> a (b c)")
                        for k in range(KC):
                            nc.tensor.matmul(
                                ps[0:twp, 0:NHG * HD],
                                xt_sb[k][:, t * KT:t * KT + twp],
                                wa_sb[k][:, 2 * NHG * HD:3 * NHG * HD],
                                start=(k == 0), stop=(k == KC - 1),
                            )
                        v6v = v6_sb[t].rearrange("a (h d) -> a h d", d=HD1)
                        if tw < 128:
                            nc.vector.memset(v6_sb[t][:, :], 0.0)
                        psv = ps[:, 0:NHG * HD].rearrange("a (h d) -> a h d", d=HD)
                        nc.vector.tensor_copy(v6v[0:tw, :, 0:HD], psv[0:tw, :, :])
                        nc.vector.memset(v6v[0:tw, :, HD:HD + 2], 1.0)

                pending = []   # deferred normalization closures

                def emit_attn_chunk(ci):
                    q0, q1 = QCHC[ci]
                    n = q1 - q0
                    items = plan[ci]
                    first_kt = items[0][0]
                    last_kt = items[-1][0]
                    for p in range(NPACK):
                        u2 = [ps8.tile([HD1, 448], F32, tag="u", name="ut", bufs=2)
                              for _ in (0, 1)]
                        sts = {}
                        ets = {}

                        def emit_s(idx):
                            kt, kw, qv0, qv1, bbox, _m = items[idx]
                            kwp = kw if kw % 2 == 0 else kw + 1
                            st = big_ps()
                            for e in (0, 1):
                                nc.tensor.matmul(
                                    st[0:kwp, e, qv0:qv1],
                                    kt_sb[p][e * 64:(e + 1) * 64,
                                             kt * KT:kt * KT + kwp],
                                    qt_sb[p][e * 64:(e + 1) * 64,
                                             q0 + qv0:q0 + qv1],
                                    start=True, stop=True,
                                )
                            sts[idx] = st

                        def emit_exp(idx):
                            kt, kw, qv0, qv1, bbox, _m = items[idx]
                            kwp = kw if kw % 2 == 0 else kw + 1
                            et = epool.tile([128, 2, 512], BF16, tag="e", name="et")
                            nc.scalar.activation(
                                et[0:kwp, :, qv0:qv1], sts[idx][0:kwp, :, qv0:qv1],
                                AF.Exp, scale=0.125,
                            )
                            ets[idx] = et

                        def emit_mask_pv(idx):
                            kt, kw, qv0, qv1, bbox, _m = items[idx]
                            kwp = kw if kw % 2 == 0 else kw + 1
                            et = ets.pop(idx)
                            sts.pop(idx)
                            if bbox is not None:
                                r0, r1, c0, c1 = bbox
                                mk = mk_sb[(ci, kt)]
                                for e in (0, 1):
                                    nc.gpsimd.tensor_mul(
                                        et[r0:r1, e, c0:c1],
                                        et[r0:r1, e, c0:c1],
                                        mk[r0:r1, 0:c1 - c0],
                                    )
                            for e in (0, 1):
                                h = 2 * p + e
                                nc.tensor.matmul(
                                    u2[e][0:HD1, qv0:qv1],
                                    v6_sb[kt][0:kwp, h * HD1:(h + 1) * HD1],
                                    et[0:kwp, e, qv0:qv1],
                                    start=(kt == first_kt), stop=(kt == last_kt),
                                    skip_group_check=(kt != first_kt),
                                )

                        # software-pipelined S -> exp -> PV (depth 2: the
                        # exp for item i has two S-pair slots of tensor
                        # time to complete before PV(i) needs it)
                        emit_s(0)
                        if len(items) > 1:
                            emit_s(1)
                        for i in range(len(items)):
                            emit_exp(i)
                            if i + 2 < len(items):
                                emit_s(i + 2)
                            emit_mask_pv(i)

                        # drain u2 quickly; defer the normalization chain
                        ubs = []
                        lns = []
                        for e in (0, 1):
                            ub = ubp.tile([HD1, 448], BF16, tag="ub", name="ub")
                            nc.vector.tensor_copy(ub[0:HD1, 0:n], u2[e][0:HD1, 0:n])
                            lnb = ubp.tile([2, 448], FP16, tag="lnb", name="lnb")
                            nc.scalar.activation(
                                lnb[0:2, 0:n], u2[e][64:66, 0:n], AF.Ln
                            )
                            ubs.append(ub)
                            lns.append(lnb)

                        def norm(p=p, ci=ci, ubs=ubs, lns=lns, n=n, q0=q0, q1=q1):
                            for e in (0, 1):
                                rb = big_ps()[0:64, 0, :]
                                nc.tensor.matmul(
                                    rb[0:64, 0:n],
                                    mones[0:2, 0:64],
                                    lns[e][0:2, 0:n],
                                    start=True, stop=True,
                                )
                                rbs = npool.tile([64, 448], BF16, tag="rbs", name="rbs")
                                nc.scalar.activation(rbs[0:64, 0:n], rb[0:64, 0:n], AF.Exp)
                                nc.gpsimd.tensor_mul(
                                    yt_sb[p][e * 64:(e + 1) * 64, q0:q1],
                                    ubs[e][0:64, 0:n],
                                    rbs[0:64, 0:n],
                                )

                        pending.append(norm)

                def emit_norms():
                    while pending:
                        pending.pop(0)()

                def emit_d_tiles(ts):
                    for t in ts:
                        tw = min(128, T - t * KT)
                        twp = tw if tw % 2 == 0 else tw + 1
                        po = big_ps()  # two 384-wide halves, one per bank
                        for hb, (n0, n1) in enumerate(((0, 384), (384, 768))):
                            for k3 in range(3):
                                nc.tensor.matmul(
                                    po[0:twp, hb, 0:384],
                                    yt_sb[k3][:, t * KT:t * KT + twp],
                                    wp_sb[k3][:, n0:n1],
                                    start=(k3 == 0), stop=(k3 == 2),
                                )
                        ot = osb.tile([128, C], BF16, tag="ot", name="ot_sb")
                        nc.vector.tensor_copy(
                            ot.rearrange("a (b c) -> a b c", c=384)[0:tw, :, :],
                            po[0:tw, :, 0:384],
                        )
                        nc.sync.dma_start(
                            out=out[t * KT:t * KT + tw, :], in_=ot[0:tw, :]
                        )

                # ---- schedule ------------------------------------------
                # D tile t needs yt chunks up to (128*(t+1)-1)//424
                d_of = {}
                for t in range(NKT):
                    d_of.setdefault(min(3, (128 * (t + 1) - 1) // 424), []).append(t)

                emit_qk_chunks([0, 1])
                emit_k_tail()
                emit_v_tiles([13, 0, 1, 2, 3])
                emit_attn_chunk(0)
                emit_qk_chunks([2, 3])
                emit_v_tiles(range(4, 13))
                emit_attn_chunk(1)
                emit_norms()          # norms for chunks 0,1
                emit_attn_chunk(2)
                emit_d_tiles(d_of[0])
                emit_d_tiles(d_of[1])
                emit_attn_chunk(3)
                emit_norms()          # norms for chunks 2,3
                emit_d_tiles(d_of[2])
                emit_d_tiles(d_of[3])

    if split:
        _split_excess_waits(nc)
    _BUILD_CACHE[key] = nc
    return nc


def _prep_inputs(x, W_attn, W_proj, mpack):
    """Per-core input maps. core c -> batch c//2, head-group c%2."""
    x = np.asarray(x, np.float32)
    W_attn = np.asarray(W_attn, np.float32)
    W_proj = np.asarray(W_proj, np.float32)
    mpack_bf = mpack.astype(NPBF16)
    in_maps = []
    xT_by_batch = []
    for b in range(B):
        xt = np.zeros((C, TP), NPBF16)
        xt[:, :T] = x[b][PERM, :].T.astype(NPBF16)
        xT_by_batch.append(xt)
    for c in range(NCORES):
        b, g = c // 2, c % 2
        cs = slice(g * NHG * HD, (g + 1) * NHG * HD)
        wa_s = np.ascontiguousarray(
            np.concatenate(
                [W_attn[:, cs], W_attn[:, C:][:, cs], W_attn[:, 2 * C:][:, cs]],
                axis=1,
            ).astype(NPBF16)
        )
        wp_s = np.ascontiguousarray(W_proj[cs, :].astype(NPBF16))
        in_maps.append(
            {"xT": xT_by_batch[b], "wa": wa_s, "wp": wp_s, "mp": mpack_bf}
        )
    return in_maps


def _run(inputs, trace=False, trace_cores=None):
    x = np.asarray(inputs["x"], np.float32)
    mask = np.asarray(inputs["mask"], bool)
    mask_perm = mask[np.ix_(PERM, PERM)]
    plan, mpack = _analyze(mask_perm)
    nc = _build(plan, mpack.shape[1])
    in_maps = _prep_inputs(x, inputs["W_attn"], inputs["W_proj"], mpack)
    res = run_bass_kernel_spmd(
        nc, in_maps, list(range(NCORES)), trace=trace, trace_cores=trace_cores
    )
    outs = [np.asarray(r["out"]).astype(np.float32) for r in res.results]
    full = np.empty((B, T, C), np.float32)
    for b in range(B):
        comb = outs[2 * b] + outs[2 * b + 1]
        full[b][PERM, :] = comb
    return full, res


def kernel(**inputs) -> np.ndarray:
    out, _ = _run(inputs)
    return out
